# revision 23
# baseline (speedup 1.0000x reference)
"""Trainium2 Bass kernel for nn_Attention_335007449334 (8-core TP attention).

Strategy: tensor-parallel over heads across 8 NeuronCores (SPMD, one program).
  - Each core owns 4 query heads + 1 kv head: wq/wk/wv column-sharded on host.
  - All matmuls run in bf16 (inputs rounded host-side / on-engine; PSUM
    accumulation stays fp32): 1 cyc/row on the PE vs 1.5 for f32r, and x is
    transposed for free by xbar DMA-transpose (2-byte dtype requirement).
  - All four weight sets (wq/wk/wv/wo) are SBUF-resident from the start
    (bf16 halves their footprint) so no mid-stream weight swap is needed.
  - Attention is computed fully transposed (scoresT [k, q]) so no
    probs-transpose is needed: softmax sums come from ones-matmuls (max-
    subtraction is skipped; score range is tiny), the causal mask is applied
    multiplicatively post-exp on the diagonal block only (off-diagonal blocks
    use restricted matmul N ranges), and normalization folds into the
    PSUM->SBUF evacuation via K=1 broadcast matmuls of the row reciprocals
    (kept f32r for precision).
  - RoPE runs on an even/odd head-dim permutation baked into the host-side
    weight column order; the K head is duplicated into swapped-half tiles so
    every DVE op is base-partition aligned. Head pairs are interleaved so
    their K=64 score matmuls pack into disjoint PE row groups.
  - The adapter cross-attention path is emitted only when tanh(gate) != 0
    (it is exactly zero otherwise); the causal fast path is used only when
    the mask matches the canonical causal pattern.
  - Per-batch attnT shards are AllGathered in bf16 (overlapped with later
    batches); wo is column-sharded and emitted per batch right after the
    batch's AllGather has had time to complete, so the final AllGather hides
    behind the previous batch's wo GEMM. Each core emits
    out^T[:, 512r:512r+512]; the host concatenates + transposes.
"""

import sys
import numpy as np
import ml_dtypes

sys.path.insert(0, "/opt/trn_rl_repo")

import concourse.bass as bass  # noqa: E402
import concourse.tile as tile  # noqa: E402
from concourse import bacc, mybir  # noqa: E402
from concourse.bass_utils import run_bass_kernel_spmd  # noqa: E402
from concourse.masks import make_identity  # noqa: E402

# If BASS_TRACE is set but this image lacks antenv.axon_hooks, bass_utils
# would crash on import; provide a stub so tracing degrades gracefully.
try:  # noqa: SIM105
    import antenv.axon_hooks  # noqa: F401
except ImportError:
    import types as _types

    try:
        import antenv  # noqa: F401

        _hooks = _types.ModuleType("antenv.axon_hooks")
        _hh = {"hook": None}
        _hooks.set_axon_ntff_profile_hook = lambda h: _hh.__setitem__("hook", h)
        _hooks.get_axon_ntff_profile_hook = lambda: _hh["hook"]
        sys.modules["antenv.axon_hooks"] = _hooks
    except ImportError:
        pass

B, S, D = 4, 512, 4096
H, HK, HD = 32, 8, 128
NCORES = 8
HL = H // NCORES  # 4 local q-heads per core
A_LEN = 64
SCALE = 1.0 / float(np.sqrt(HD))

F32 = mybir.dt.float32
F32R = mybir.dt.float32r
BF16 = mybir.dt.bfloat16

_cache = {}
last_result = None


def _host_prep(inputs):
    x = np.asarray(inputs["x"], np.float32).reshape(B * S, D)
    adapter = np.asarray(inputs["adapter"], np.float32).reshape(B * A_LEN, D)
    mask = np.asarray(inputs["mask"], np.float32)[0, 0]
    cos = np.asarray(inputs["freqs_cos"], np.float32)
    sin = np.asarray(inputs["freqs_sin"], np.float32)
    wq = np.asarray(inputs["wq"], np.float32)
    wk = np.asarray(inputs["wk"], np.float32)
    wv = np.asarray(inputs["wv"], np.float32)
    wo = np.asarray(inputs["wo"], np.float32)
    gate = np.asarray(inputs["gate"], np.float32)[0, :, 0, 0]
    tg = np.tanh(gate).astype(np.float32)

    canonical = np.where(
        np.tril(np.ones((S, S), dtype=bool)), np.float32(0.0), np.float32(-1e9)
    ).astype(np.float32)
    causal = bool(np.array_equal(mask, canonical))
    adapter_skip = bool(np.all(tg == 0.0))

    cosT = np.ascontiguousarray(cos.T)  # [64, S]
    sinT = np.ascontiguousarray(sin.T)
    C2 = np.ascontiguousarray(np.concatenate([cosT, cosT], axis=0))
    S2 = np.ascontiguousarray(np.concatenate([sinT, sinT], axis=0))
    S2a = np.ascontiguousarray(np.concatenate([-sinT, sinT], axis=0))
    S2b = np.ascontiguousarray(np.concatenate([sinT, -sinT], axis=0))

    if causal:
        em = np.exp(mask[0:128, 0:128].T)
    else:
        em = np.exp(mask.T)  # [k, q]
    em = np.ascontiguousarray(em.astype(ml_dtypes.bfloat16))

    ev = np.arange(0, HD, 2)
    od = np.arange(1, HD, 2)

    xb = np.ascontiguousarray(x.T).astype(ml_dtypes.bfloat16)  # [D, B*S]
    in_maps = []
    for r in range(NCORES):
        heads = [4 * r + i for i in range(HL)]
        cols = []
        for p in range(HL // 2):
            h0, h1 = heads[2 * p], heads[2 * p + 1]
            cols.append(np.concatenate([h0 * HD + ev, h1 * HD + ev]))
            cols.append(np.concatenate([h0 * HD + od, h1 * HD + od]))
        wq_r = np.ascontiguousarray(wq[:, np.concatenate(cols)])
        ka_cols = np.concatenate([r * HD + ev, r * HD + od])
        wk_r = np.ascontiguousarray(wk[:, ka_cols])
        wv_r = np.ascontiguousarray(wv[:, r * HD : (r + 1) * HD])
        wo_r = np.ascontiguousarray(wo[:, 512 * r : 512 * (r + 1)])
        m = dict(
            x=xb,
            wq_r=wq_r.astype(ml_dtypes.bfloat16),
            wk_r=wk_r.astype(ml_dtypes.bfloat16),
            wv_r=wv_r.astype(ml_dtypes.bfloat16),
            wo_r=wo_r.astype(ml_dtypes.bfloat16),
            C2=C2, S2=S2, S2a=S2a, S2b=S2b, em=em,
        )
        if not adapter_skip:
            m["adapter"] = np.ascontiguousarray(adapter.T).astype(ml_dtypes.bfloat16)
            m["tg4"] = np.ascontiguousarray(tg[4 * r : 4 * r + 4].reshape(1, 4))
        in_maps.append(m)
    return in_maps, causal, adapter_skip


def _build(causal, adapter_skip):
    nc = bacc.Bacc(trn_type="TRN2", num_devices=NCORES)

    x_d = nc.dram_tensor("x", [D, B * S], BF16, kind="ExternalInput")  # x^T
    if not adapter_skip:
        ad = nc.dram_tensor("adapter", [D, B * A_LEN], BF16, kind="ExternalInput")
    wq_r = nc.dram_tensor("wq_r", [D, 512], BF16, kind="ExternalInput")
    wk_r = nc.dram_tensor("wk_r", [D, 128], BF16, kind="ExternalInput")
    wv_r = nc.dram_tensor("wv_r", [D, 128], BF16, kind="ExternalInput")
    wo_r = nc.dram_tensor("wo_r", [D, 512], BF16, kind="ExternalInput")
    c2_d = nc.dram_tensor("C2", [128, S], F32, kind="ExternalInput")
    s2_d = nc.dram_tensor("S2", [128, S], F32, kind="ExternalInput")
    s2a_d = nc.dram_tensor("S2a", [128, S], F32, kind="ExternalInput")
    s2b_d = nc.dram_tensor("S2b", [128, S], F32, kind="ExternalInput")
    em_shape = [128, 128] if causal else [S, S]
    em_d = nc.dram_tensor("em", em_shape, BF16, kind="ExternalInput")
    if not adapter_skip:
        tg_d = nc.dram_tensor("tg4", [1, HL], F32, kind="ExternalInput")
    out_r = nc.dram_tensor("out_r", [512, B * S], F32, kind="ExternalOutput")

    rg = [list(range(NCORES))]

    with tile.TileContext(nc) as tc:
        with (
            tc.tile_pool(name="const", bufs=1) as constp,
            tc.tile_pool(name="wres", bufs=1) as wres,
            tc.tile_pool(name="xts", bufs=16) as xstream,
            tc.tile_pool(name="rtmp", bufs=1) as rtmp,
            tc.tile_pool(name="batp", bufs=2) as batp,
            tc.tile_pool(name="dram", bufs=1, space="DRAM") as dram,
        ):
            # ---- resident weights (all four, bf16, gpsimd queue) ----
            # order: group 0 of each QKV weight first so qkv0 can start,
            # then the rest; wow last (first needed ~200us in).
            wqw = wres.tile([128, 32, 512], BF16)
            kaw = wres.tile([128, 32, 128], BF16)
            wvw = wres.tile([128, 32, 128], BF16)
            wow = wres.tile([128, 32, 512], BF16)
            for g in range(8):
                gs = slice(4 * g, 4 * (g + 1))
                nc.gpsimd.dma_start(
                    kaw[:, gs, :],
                    wk_r[:].rearrange("(kt p) c -> p kt c", p=128)[:, gs, :],
                )
                nc.gpsimd.dma_start(
                    wvw[:, gs, :],
                    wv_r[:].rearrange("(kt p) c -> p kt c", p=128)[:, gs, :],
                )
                nc.gpsimd.dma_start(
                    wqw[:, gs, :],
                    wq_r[:].rearrange("(kt p) c -> p kt c", p=128)[:, gs, :],
                )
            # ---- constants (gpsimd queue: keeps sync/scalar free for xT;
            # needed first at rope time, ~50us in) ----
            ident = constp.tile([128, 128], F32)
            make_identity(nc, ident[:])
            ident_b = constp.tile([128, 128], BF16)
            nc.vector.tensor_copy(ident_b[:], ident[:])
            ones_f = constp.tile([128, 1], F32)
            nc.vector.memset(ones_f[:], 1.0)
            ones_b = constp.tile([128, 1], BF16)
            nc.vector.tensor_copy(ones_b[:], ones_f[:])
            ones1f = constp.tile([1, 128], F32)
            nc.vector.memset(ones1f[:], 1.0)
            ones1r = constp.tile([1, 128], F32R)
            nc.vector.tensor_copy(ones1r[:], ones1f[:])
            c2 = constp.tile([128, S], F32)
            nc.gpsimd.dma_start(c2[:], c2_d[:])
            s2 = constp.tile([128, S], F32)
            nc.gpsimd.dma_start(s2[:], s2_d[:])
            s2a = constp.tile([128, S], F32)
            nc.gpsimd.dma_start(s2a[:], s2a_d[:])
            s2b = constp.tile([128, S], F32)
            nc.gpsimd.dma_start(s2b[:], s2b_d[:])
            em_sb = constp.tile(em_shape if causal else [128, 4, S], BF16)
            if causal:
                nc.gpsimd.dma_start(em_sb[:], em_d[:])
            else:
                nc.gpsimd.dma_start(
                    em_sb[:], em_d[:].rearrange("(kc p) q -> p kc q", p=128)
                )
            if not adapter_skip:
                tg4 = constp.tile([1, HL], F32)
                nc.gpsimd.dma_start(tg4[:], tg_d[:])
            # wo weights last on the queue: first needed ~200us in
            for g in range(4):
                gs = slice(8 * g, 8 * (g + 1))
                nc.gpsimd.dma_start(
                    wow[:, gs, :],
                    wo_r[:].rearrange("(kt p) c -> p kt c", p=128)[:, gs, :],
                )

            # ---- adapter transpose + projections (generic path only) ----
            if not adapter_skip:
                with (
                    tc.tile_pool(name="adp", bufs=3) as adp,
                    tc.tile_pool(name="adps", bufs=2, space="PSUM") as adps,
                ):
                    akt = constp.tile([128, B, A_LEN], BF16)
                    aktb = constp.tile([128, B, A_LEN], BF16)
                    avt = adp.tile([128, B * A_LEN], BF16)
                    pk = adps.tile([128, 256], F32, name="pk")
                    pv = adps.tile([128, 256], F32, name="pv")
                    for kt in range(32):
                        adt_t = adp.tile([128, 256], BF16, tag="adt")
                        nc.sync.dma_start(
                            adt_t[:], ad[128 * kt : 128 * (kt + 1), :]
                        )
                        nc.tensor.matmul(
                            pk[:], kaw[:, kt, :], adt_t[:],
                            start=(kt == 0), stop=(kt == 31),
                        )
                        nc.tensor.matmul(
                            pv[:], wvw[:, kt, :], adt_t[:],
                            start=(kt == 0), stop=(kt == 31),
                        )
                    nc.scalar.copy(akt[:].rearrange("p b a -> p (b a)"), pk[:])
                    nc.scalar.copy(avt[:], pv[:])
                    # aKTB = swapped halves of aKT
                    nc.sync.dma_start(
                        aktb[0:64, :, :].bitcast(F32), akt[64:128, :, :].bitcast(F32)
                    )
                    nc.sync.dma_start(
                        aktb[64:128, :, :].bitcast(F32), akt[0:64, :, :].bitcast(F32)
                    )
                    # aV token-major per batch
                    av_sb = constp.tile([64, B, 128], BF16)
                    for b in range(B):
                        pav = adps.tile([64, 128], BF16)
                        nc.tensor.transpose(
                            pav[:], avt[:, 64 * b : 64 * (b + 1)], ident_b[:]
                        )
                        nc.scalar.copy(av_sb[:, b, :], pav[:])

            # ---- per-batch attnT shards in DRAM + AllGather targets ----
            at_in = []
            at_full = []
            for b in range(B):
                at_in.append(dram.tile([512, S], BF16, name=f"at_in{b}"))
                at_full.append(
                    dram.tile([D, S], BF16, addr_space="Shared", name=f"at_full{b}")
                )

            qkv_state = {}
            xt_prefetch = {}

            def prefetch_xt(b, upto):
                """Issue xt DMA loads for batch b, kt < upto (from attn blocks,
                while sync/scalar queues are idle)."""
                if b >= B:
                    return
                for kt in range(upto):
                    if (b, kt) in xt_prefetch:
                        continue
                    xt_t = xstream.tile([128, 512], BF16, tag="xt")
                    eng = nc.sync if kt % 2 == 0 else nc.scalar
                    eng.dma_start(
                        xt_t[:],
                        x_d[128 * kt : 128 * (kt + 1), 512 * b : 512 * (b + 1)],
                    )
                    xt_prefetch[(b, kt)] = xt_t

            def emit_qkv(b):
                bat = batp
                with (
                    tc.tile_pool(name=f"tps{b}", bufs=2, space="PSUM") as tps,
                    tc.tile_pool(name=f"qkvps{b}", bufs=1, space="PSUM") as qkvps,
                ):
                    q_ps = [
                        qkvps.tile([128, 512], F32, name=f"qps{m}") for m in range(4)
                    ]
                    k_ps = qkvps.tile([128, 512], F32)
                    v_ps = qkvps.tile([128, 512], F32)

                    def emit_mms(kt, xt_t):
                        st, sp = (kt == 0), (kt == 31)
                        for m in range(4):
                            nc.tensor.matmul(
                                q_ps[m][:], wqw[:, kt, 128 * m : 128 * (m + 1)],
                                xt_t[:], start=st, stop=sp,
                            )
                        nc.tensor.matmul(k_ps[:], kaw[:, kt, :], xt_t[:], start=st, stop=sp)
                        nc.tensor.matmul(v_ps[:], wvw[:, kt, :], xt_t[:], start=st, stop=sp)

                    # xT streamed by xbar DMA-transpose, 2 kt deep, on
                    # alternating sync/scalar queues.
                    pend = []
                    for kt in range(32):
                        xt_t = xt_prefetch.pop((b, kt), None)
                        if xt_t is None:
                            xt_t = xstream.tile([128, 512], BF16, tag="xt")
                            eng = nc.sync if kt % 2 == 0 else nc.scalar
                            eng.dma_start(
                                xt_t[:],
                                x_d[128 * kt : 128 * (kt + 1), 512 * b : 512 * (b + 1)],
                            )
                        pend.append((kt, xt_t))
                        if len(pend) > 2:
                            emit_mms(*pend.pop(0))
                    for p in pend:
                        emit_mms(*p)

                    # RoPE on Q pair-blocks: rqA = QA*C2 - QB*S2 ; rqB = QA*S2 + QB*C2
                    rqa, rqb = [], []
                    for p in range(2):
                        qa, qb = q_ps[2 * p], q_ps[2 * p + 1]
                        t1 = rtmp.tile([128, S], F32, tag="t1")
                        t2 = rtmp.tile([128, S], F32, tag="t2")
                        ra = bat.tile([128, S], BF16, tag=f"rqa{p}")
                        rb = bat.tile([128, S], BF16, tag=f"rqb{p}")
                        nc.vector.tensor_mul(t1[:], qa[:], c2[:])
                        nc.vector.tensor_mul(t2[:], qb[:], s2[:])
                        nc.vector.tensor_sub(ra[:], t1[:], t2[:])
                        nc.vector.tensor_mul(t1[:], qa[:], s2[:])
                        nc.vector.tensor_mul(t2[:], qb[:], c2[:])
                        nc.vector.tensor_add(rb[:], t1[:], t2[:])
                        rqa.append(ra)
                        rqb.append(rb)
                    # K: ka/kb swap-duplicate, then rope
                    ka_f = rtmp.tile([128, S], F32, tag="ka_f")
                    nc.scalar.copy(ka_f[:], k_ps[:])
                    kb_f = rtmp.tile([128, S], F32, tag="kb_f")
                    nc.scalar.dma_start(kb_f[0:64, :], ka_f[64:128, :])
                    nc.scalar.dma_start(kb_f[64:128, :], ka_f[0:64, :])
                    t1 = rtmp.tile([128, S], F32, tag="t1")
                    t2 = rtmp.tile([128, S], F32, tag="t2")
                    rka = bat.tile([128, S], BF16, tag="rka")
                    rkb = bat.tile([128, S], BF16, tag="rkb")
                    nc.vector.tensor_mul(t1[:], ka_f[:], c2[:])
                    nc.vector.tensor_mul(t2[:], kb_f[:], s2a[:])
                    nc.vector.tensor_add(rka[:], t1[:], t2[:])
                    nc.vector.tensor_mul(t1[:], kb_f[:], c2[:])
                    nc.vector.tensor_mul(t2[:], ka_f[:], s2b[:])
                    nc.vector.tensor_add(rkb[:], t1[:], t2[:])
                    # V: token-major (bf16 PE transposes)
                    vt_f = rtmp.tile([128, S], BF16, tag="vt_f")
                    nc.scalar.copy(vt_f[:], v_ps[:])
                    v_sb = bat.tile([128, 4, 128], BF16, tag="v_sb")
                    for kc in range(4):
                        pv2 = tps.tile([128, 128], BF16, tag="pst")
                        nc.tensor.transpose(
                            pv2[:], vt_f[:, 128 * kc : 128 * (kc + 1)], ident_b[:]
                        )
                        nc.scalar.copy(v_sb[:, kc, :], pv2[:])
                qkv_state[b] = (rqa, rqb, rka, rkb, v_sb)

            def emit_attn(b, wo_b=None, pf_b=None):
                if adapter_skip:
                    emit_attn_fast(b, wo_b, pf_b)
                else:
                    emit_attn_generic(b)
                    if wo_b is not None:
                        emit_wo(wo_b)

            def emit_attn_fast(b, wo_b=None, pf_b=None):
                """Head-pair-interleaved attention, with the wo GEMM of an
                earlier batch (already AllGathered) chunk-interleaved into the
                dependency gaps so the PE never idles.  Emission order per
                pair is scores(p0) / smav(p0) / scores(p1) / norm(p0) /
                smav(p1) / norm(p1): each norm chain hides behind the other
                pair's matmuls."""
                rqa, rqb, rka, rkb, v_sb = qkv_state.pop(b)
                wo_chunks = []
                with (
                    tc.tile_pool(name=f"ex{b}", bufs=2) as exp_pool,
                    tc.tile_pool(name=f"au{b}", bufs=2) as aup,
                    tc.tile_pool(name=f"smp{b}", bufs=2) as smp,
                    tc.tile_pool(name=f"rhs{b}", bufs=1) as rhsp,
                    tc.tile_pool(name=f"woo{b}", bufs=2) as woop,
                    tc.tile_pool(name=f"scps{b}", bufs=2, space="PSUM") as scps,
                    tc.tile_pool(name=f"avps{b}", bufs=2, space="PSUM") as avps,
                    tc.tile_pool(name=f"smps{b}", bufs=2, space="PSUM") as smps,
                    tc.tile_pool(name=f"wops{b}", bufs=2, space="PSUM") as wops,
                ):
                    if wo_b is not None:
                        wo_chunks = make_wo_chunks(wo_b, rhsp, woop, wops)
                    if pf_b is not None:
                        prefetch_xt(pf_b, 4)

                    import os as _os2
                    _ilv_mid = _os2.environ.get("KERNEL_WO_MID", "1") == "1"

                    def wo_step(n, force=False):
                        if not (_ilv_mid or force):
                            return
                        for _ in range(n):
                            if wo_chunks:
                                wo_chunks.pop(0)()

                    def scores(pp):
                        rqe, rqo = rqa[pp], rqb[pp]
                        sc_ps = {0: [], 1: []}
                        for kc in range(4):
                            qlo = 128 * kc if causal else 0
                            for hh in range(2):
                                sc_ps[hh].append(
                                    scps.tile([128, S], F32, tag="sc",
                                              name=f"sc{hh}")
                                )
                            for hh, which in ((0, "e"), (1, "e"), (0, "o"), (1, "o")):
                                beta = 64 * hh
                                sl = slice(beta, beta + 64)
                                if which == "e":
                                    lh = (rka if hh == 0 else rkb)
                                    rh = rqe
                                else:
                                    lh = (rkb if hh == 0 else rka)
                                    rh = rqo
                                nc.tensor.matmul(
                                    sc_ps[hh][kc][:, qlo:S],
                                    lh[sl, 128 * kc : 128 * (kc + 1)],
                                    rh[sl, qlo:S],
                                    start=(which == "e"), stop=(which == "o"),
                                )
                        return sc_ps

                    def smav(pp, sc_ps):
                        expT = {}
                        av_p = {}
                        sm2 = smps.tile([128, S], F32, tag="sm4", name=f"sm4p{pp}")
                        for hh in range(2):
                            expT[hh] = exp_pool.tile(
                                [128, 4, S], BF16, tag="expT", name=f"expT{hh}"
                            )
                            av_p[hh] = avps.tile(
                                [128, S], F32, tag="av", name=f"av{hh}"
                            )
                        for kc in range(4):
                            qlo = 128 * kc if causal else 0
                            for hh in range(2):
                                nc.scalar.activation(
                                    expT[hh][:, kc, qlo:S], sc_ps[hh][kc][:, qlo:S],
                                    func=mybir.ActivationFunctionType.Exp,
                                    scale=SCALE,
                                )
                                if causal:
                                    nc.vector.tensor_mul(
                                        expT[hh][:, kc, qlo : qlo + 128],
                                        expT[hh][:, kc, qlo : qlo + 128],
                                        em_sb[:],
                                    )
                                else:
                                    nc.vector.tensor_mul(
                                        expT[hh][:, kc, :],
                                        expT[hh][:, kc, :],
                                        em_sb[:, kc, :],
                                    )
                                row = 64 * hh
                                nc.tensor.matmul(
                                    sm2[row : row + 1, qlo:S], ones_b[:, 0:1],
                                    expT[hh][:, kc, qlo:S],
                                    start=(kc == 0), stop=(kc == 3),
                                )
                                nc.tensor.matmul(
                                    av_p[hh][:, qlo:S], v_sb[:, kc, :],
                                    expT[hh][:, kc, qlo:S],
                                    start=(kc == 0), stop=(kc == 3),
                                )
                        return av_p, sm2

                    def norm(pp, avsm):
                        av_p, sm2 = avsm
                        for hh in range(2):
                            h = 2 * pp + hh
                            row = 64 * hh
                            au = aup.tile([128, S], F32, tag="attnU")
                            nc.scalar.copy(au[:], av_p[hh][:])
                            smtr = smp.tile([1, 2, S], F32, tag="smt")
                            nc.scalar.copy(smtr[:, 0, :], sm2[row : row + 1, :])
                            nc.vector.reciprocal_approx_fast(
                                smtr[:, 1, :], smtr[:, 0, :]
                            )
                            smrr = smp.tile([1, S], F32R, tag="smrr")
                            nc.vector.tensor_copy(smrr[:], smtr[:, 1, :])
                            rb_ps = avps.tile([128, S], F32, tag="av", name="rb_ps")
                            nc.tensor.matmul(
                                rb_ps[:], ones1r[0:1, :], smrr[0:1, :],
                                start=True, stop=True,
                            )
                            at_n = aup.tile([128, S], BF16, tag="at_n")
                            nc.vector.tensor_mul(at_n[:], au[:], rb_ps[:])
                            nc.gpsimd.dma_start(
                                at_in[b][128 * h : 128 * (h + 1), :], at_n[:]
                            )

                    sc0 = scores(0)
                    wo_step(1)
                    av0 = smav(0, sc0)
                    wo_step(1)
                    sc1 = scores(1)
                    norm(0, av0)
                    wo_step(1)
                    av1 = smav(1, sc1)
                    wo_step(1)
                    norm(1, av1)
                    nc.gpsimd.collective_compute(
                        "AllGather", mybir.AluOpType.bypass, replica_groups=rg,
                        ins=[at_in[b][:]], outs=[at_full[b][:]],
                    )
                    wo_step(len(wo_chunks), force=True)

            def emit_attn_generic(b):
                rqa, rqb, rka, rkb, v_sb = qkv_state.pop(b)
                with (
                    tc.tile_pool(name=f"ex{b}", bufs=1) as exp_pool,
                    tc.tile_pool(name=f"au{b}", bufs=1) as aup,
                    tc.tile_pool(name=f"smp{b}", bufs=1) as smp,
                    tc.tile_pool(name=f"scps{b}", bufs=2, space="PSUM") as scps,
                    tc.tile_pool(name=f"avps{b}", bufs=2, space="PSUM") as avps,
                    tc.tile_pool(name=f"smps{b}", bufs=1, space="PSUM") as smps,
                    tc.tile_pool(name=f"ascps{b}", bufs=1, space="PSUM") as ascps,
                ):
                    for h in range(HL):
                        p, beta = h // 2, 64 * (h % 2)
                        sl = slice(beta, beta + 64)
                        rqe, rqo = rqa[p], rqb[p]
                        rke_t = rka if beta == 0 else rkb
                        rko_t = rkb if beta == 0 else rka
                        expT = exp_pool.tile([128, 4, S], BF16, tag="expT")
                        av_p = avps.tile([128, S], F32, tag="av")
                        sm_p = smps.tile([1, S], F32, tag="sm")
                        sc_ps = []
                        for kc in range(4):
                            qlo = 128 * kc if causal else 0
                            sc_p = scps.tile([128, S], F32, tag="sc")
                            sc_ps.append(sc_p)
                            nc.tensor.matmul(
                                sc_p[:, qlo:S],
                                rke_t[sl, 128 * kc : 128 * (kc + 1)],
                                rqe[sl, qlo:S],
                                start=True, stop=False,
                            )
                            nc.tensor.matmul(
                                sc_p[:, qlo:S],
                                rko_t[sl, 128 * kc : 128 * (kc + 1)],
                                rqo[sl, qlo:S],
                                start=False, stop=True,
                            )
                        for kc in range(4):
                            qlo = 128 * kc if causal else 0
                            nc.scalar.activation(
                                expT[:, kc, qlo:S], sc_ps[kc][:, qlo:S],
                                func=mybir.ActivationFunctionType.Exp, scale=SCALE,
                            )
                            if causal:
                                nc.vector.tensor_mul(
                                    expT[:, kc, qlo : qlo + 128],
                                    expT[:, kc, qlo : qlo + 128],
                                    em_sb[:],
                                )
                            else:
                                nc.vector.tensor_mul(
                                    expT[:, kc, :],
                                    expT[:, kc, :],
                                    em_sb[:, kc, :],
                                )
                            nc.tensor.matmul(
                                sm_p[0:1, qlo:S], ones_b[:, 0:1],
                                expT[:, kc, qlo:S],
                                start=(kc == 0), stop=(kc == 3),
                            )
                            nc.tensor.matmul(
                                av_p[:, qlo:S], v_sb[:, kc, :],
                                expT[:, kc, qlo:S],
                                start=(kc == 0), stop=(kc == 3),
                            )
                        au = aup.tile([128, S], F32, tag="attnU")
                        nc.scalar.copy(au[:], av_p[:])
                        smt = smp.tile([1, S], F32, tag="smt")
                        nc.scalar.copy(smt[:], sm_p[0:1, :])
                        smr = smp.tile([1, S], F32, tag="smr")
                        nc.vector.reciprocal_approx_fast(smr[:], smt[:])
                        rb_ps = avps.tile([128, S], F32, tag="av", name="rb_ps")
                        nc.tensor.matmul(
                            rb_ps[:], ones1f[0:1, :], smr[0:1, :],
                            start=True, stop=True,
                        )
                        at_n = aup.tile([128, S], BF16, tag="at_n")
                        asc_p = ascps.tile([64, S], F32, tag="asc")
                        ke_src = akt if beta == 0 else aktb
                        ko_src = aktb if beta == 0 else akt
                        nc.tensor.matmul(
                            asc_p[:], ke_src[sl, b, :], rqe[sl, :],
                            start=True, stop=False,
                        )
                        nc.tensor.matmul(
                            asc_p[:], ko_src[sl, b, :], rqo[sl, :],
                            start=False, stop=True,
                        )
                        a_expT = exp_pool.tile([64, S], BF16, tag="a_expT")
                        nc.scalar.activation(
                            a_expT[:], asc_p[:],
                            func=mybir.ActivationFunctionType.Exp, scale=SCALE,
                        )
                        asm_p = smps.tile([1, S], F32, tag="asm")
                        nc.tensor.matmul(
                            asm_p[0:1, :], ones_b[0:64, 0:1], a_expT[:],
                            start=True, stop=True,
                        )
                        aav_p = avps.tile([128, S], F32, tag="av")
                        nc.tensor.matmul(
                            aav_p[:], av_sb[:, b, :], a_expT[:],
                            start=True, stop=True,
                        )
                        aau = aup.tile([128, S], F32, tag="a_attnU")
                        nc.scalar.copy(aau[:], aav_p[:])
                        asmt = aup.tile([1, S], F32, tag="asmt")
                        nc.scalar.copy(asmt[:], asm_p[0:1, :])
                        asmr = aup.tile([1, S], F32, tag="asmr")
                        nc.vector.reciprocal_approx_fast(asmr[:], asmt[:])
                        nc.vector.tensor_scalar_mul(
                            asmr[:], asmr[:], tg4[0:1, h : h + 1]
                        )
                        arb_ps = avps.tile([128, S], F32, tag="av", name="arb_ps")
                        nc.tensor.matmul(
                            arb_ps[:], ones1f[0:1, :], asmr[0:1, :],
                            start=True, stop=True,
                        )
                        t_m = aup.tile([128, S], F32, tag="t_m")
                        nc.vector.tensor_mul(t_m[:], au[:], rb_ps[:])
                        t_a = aup.tile([128, S], F32, tag="t_a")
                        nc.vector.tensor_mul(t_a[:], aau[:], arb_ps[:])
                        nc.vector.tensor_add(at_n[:], t_m[:], t_a[:])
                        nc.gpsimd.dma_start(
                            at_in[b][128 * h : 128 * (h + 1), :], at_n[:]
                        )

                nc.gpsimd.collective_compute(
                    "AllGather", mybir.AluOpType.bypass, replica_groups=rg,
                    ins=[at_in[b][:]], outs=[at_full[b][:]],
                )

            def make_wo_chunks(b, rhsp, woop, wops):
                """wo GEMM for batch b as a list of chunk closures: all rhs
                resident up-front, then per-m 32-kt accumulation in a single
                PSUM bank (double-buffered across m), evacuated per m."""
                rhs_t = rhsp.tile([128, 16, 2, 512], BF16, name=f"worhs{b}")
                for kp in range(16):
                    eng = nc.sync if kp % 2 == 0 else nc.scalar
                    eng.dma_start(
                        rhs_t[:, kp, :, :],
                        at_full[b][
                            256 * kp : 256 * (kp + 1), :
                        ].rearrange("(two p) t -> p two t", p=128),
                    )
                state = {}

                def mk_mm(m, kp_lo, kp_hi):
                    def go():
                        if m not in state:
                            state[m] = wops.tile(
                                [128, 512], F32, tag="ops", name=f"ops{m}_{b}"
                            )
                        o_ps = state[m]
                        for kp in range(kp_lo, kp_hi):
                            for j in range(2):
                                kt = 2 * kp + j
                                nc.tensor.matmul(
                                    o_ps[:],
                                    wow[:, kt, 128 * m : 128 * (m + 1)],
                                    rhs_t[:, kp, j, :],
                                    start=(kt == 0), stop=(kt == 31),
                                )
                    return go

                def mk_evac(m):
                    def go():
                        o_ps = state.pop(m)
                        osb = woop.tile([128, 512], F32, tag="osb")
                        nc.scalar.copy(osb[:], o_ps[:])
                        nc.sync.dma_start(
                            out_r[
                                128 * m : 128 * (m + 1),
                                512 * b : 512 * (b + 1),
                            ],
                            osb[:],
                        )
                    return go

                chunks = []
                for m in range(4):
                    for kp_lo in range(0, 16, 4):
                        chunks.append(mk_mm(m, kp_lo, kp_lo + 4))
                    chunks.append(mk_evac(m))
                return chunks

            def emit_wo(b):
                with (
                    tc.tile_pool(name=f"rhs{b}w", bufs=1) as rhsp,
                    tc.tile_pool(name=f"woo{b}w", bufs=2) as woop,
                    tc.tile_pool(name=f"wops{b}w", bufs=2, space="PSUM") as wops,
                ):
                    for ch in make_wo_chunks(b, rhsp, woop, wops):
                        ch()

            emit_qkv(0)
            emit_qkv(1)
            emit_attn(0, pf_b=2)
            emit_qkv(2)
            emit_attn(1, pf_b=3)
            emit_qkv(3)
            import os as _os
            if _os.environ.get("KERNEL_WO_ILV", "1") == "1":
                emit_attn(2, wo_b=0)
                emit_attn(3, wo_b=1)
                emit_wo(2)
                emit_wo(3)
            else:
                emit_attn(2)
                emit_attn(3)
                emit_wo(0)
                emit_wo(1)
                emit_wo(2)
                emit_wo(3)

    nc.compile()
    return nc


def kernel(**inputs) -> np.ndarray:
    in_maps, causal, adapter_skip = _host_prep(inputs)
    key = (causal, adapter_skip)
    if key not in _cache:
        _cache[key] = _build(causal, adapter_skip)
    nc = _cache[key]
    res = run_bass_kernel_spmd(nc, in_maps, core_ids=list(range(NCORES)))
    global last_result
    last_result = res
    out = np.empty((B * S, D), np.float32)
    for r in range(NCORES):
        out[:, 512 * r : 512 * (r + 1)] = res.results[r]["out_r"].T
    return out.reshape(B, S, D)


if __name__ == "__main__":
    rng = np.random.default_rng(0)
    demo = {
        "x": rng.standard_normal((B, S, D), dtype=np.float32),
        "adapter": rng.standard_normal((B, A_LEN, D), dtype=np.float32),
        "mask": np.where(
            np.tril(np.ones((S, S), dtype=bool)), 0.0, -1e9
        ).astype(np.float32)[None, None],
        "freqs_cos": rng.random((S, 64), dtype=np.float32),
        "freqs_sin": rng.random((S, 64), dtype=np.float32),
        "wq": (rng.standard_normal((D, H * HD), dtype=np.float32) * 0.02),
        "wk": (rng.standard_normal((D, HK * HD), dtype=np.float32) * 0.02),
        "wv": (rng.standard_normal((D, HK * HD), dtype=np.float32) * 0.02),
        "wo": (rng.standard_normal((H * HD, D), dtype=np.float32) * 0.02),
        "gate": np.zeros((1, H, 1, 1), np.float32),
    }
    o = kernel(**demo)
    print("kernel ran, out shape", o.shape)


# revision 27
# speedup vs baseline: 1.0482x; 1.0482x over previous
"""Trainium2 Bass kernel for nn_Attention_335007449334 (8-core TP attention).

Strategy: tensor-parallel over heads across 8 NeuronCores (SPMD, one program).
  - Each core owns 4 query heads + 1 kv head: wq/wk/wv column-sharded on host.
  - All matmuls run in bf16 (inputs rounded host-side / on-engine; PSUM
    accumulation stays fp32): 1 cyc/row on the PE vs 1.5 for f32r, and x is
    transposed for free by xbar DMA-transpose (2-byte dtype requirement).
  - All four weight sets (wq/wk/wv/wo) are SBUF-resident from the start
    (bf16 halves their footprint) so no mid-stream weight swap is needed.
  - Attention is computed fully transposed (scoresT [k, q]) so no
    probs-transpose is needed: softmax sums come from ones-matmuls (max-
    subtraction is skipped; score range is tiny), the causal mask is applied
    multiplicatively post-exp on the diagonal block only (off-diagonal blocks
    use restricted matmul N ranges), and normalization folds into the
    PSUM->SBUF evacuation via K=1 broadcast matmuls of the row reciprocals
    (kept f32r for precision).
  - RoPE runs on an even/odd head-dim permutation baked into the host-side
    weight column order; the K head is duplicated into swapped-half tiles so
    every DVE op is base-partition aligned. Head pairs are interleaved so
    their K=64 score matmuls pack into disjoint PE row groups.
  - The adapter cross-attention path is emitted only when tanh(gate) != 0
    (it is exactly zero otherwise); the causal fast path is used only when
    the mask matches the canonical causal pattern.
  - Per-batch attnT shards are AllGathered in bf16 (overlapped with later
    batches); wo is column-sharded and emitted per batch right after the
    batch's AllGather has had time to complete, so the final AllGather hides
    behind the previous batch's wo GEMM. Each core emits
    out^T[:, 512r:512r+512]; the host concatenates + transposes.
"""

import sys
import numpy as np
import ml_dtypes

sys.path.insert(0, "/opt/trn_rl_repo")

import concourse.bass as bass  # noqa: E402
import concourse.tile as tile  # noqa: E402
from concourse import bacc, mybir  # noqa: E402
from concourse.bass_utils import run_bass_kernel_spmd  # noqa: E402
from concourse.masks import make_identity  # noqa: E402

# If BASS_TRACE is set but this image lacks antenv.axon_hooks, bass_utils
# would crash on import; provide a stub so tracing degrades gracefully.
try:  # noqa: SIM105
    import antenv.axon_hooks  # noqa: F401
except ImportError:
    import types as _types

    try:
        import antenv  # noqa: F401

        _hooks = _types.ModuleType("antenv.axon_hooks")
        _hh = {"hook": None}
        _hooks.set_axon_ntff_profile_hook = lambda h: _hh.__setitem__("hook", h)
        _hooks.get_axon_ntff_profile_hook = lambda: _hh["hook"]
        sys.modules["antenv.axon_hooks"] = _hooks
    except ImportError:
        pass

B, S, D = 4, 512, 4096
H, HK, HD = 32, 8, 128
NCORES = 8
HL = H // NCORES  # 4 local q-heads per core
A_LEN = 64
SCALE = 1.0 / float(np.sqrt(HD))

F32 = mybir.dt.float32
F32R = mybir.dt.float32r
BF16 = mybir.dt.bfloat16

_cache = {}
last_result = None


def _host_prep(inputs):
    x = np.asarray(inputs["x"], np.float32).reshape(B * S, D)
    adapter = np.asarray(inputs["adapter"], np.float32).reshape(B * A_LEN, D)
    mask = np.asarray(inputs["mask"], np.float32)[0, 0]
    cos = np.asarray(inputs["freqs_cos"], np.float32)
    sin = np.asarray(inputs["freqs_sin"], np.float32)
    wq = np.asarray(inputs["wq"], np.float32)
    wk = np.asarray(inputs["wk"], np.float32)
    wv = np.asarray(inputs["wv"], np.float32)
    wo = np.asarray(inputs["wo"], np.float32)
    gate = np.asarray(inputs["gate"], np.float32)[0, :, 0, 0]
    tg = np.tanh(gate).astype(np.float32)

    canonical = np.where(
        np.tril(np.ones((S, S), dtype=bool)), np.float32(0.0), np.float32(-1e9)
    ).astype(np.float32)
    causal = bool(np.array_equal(mask, canonical))
    adapter_skip = bool(np.all(tg == 0.0))

    cosT = np.ascontiguousarray(cos.T)  # [64, S]
    sinT = np.ascontiguousarray(sin.T)
    C2 = np.ascontiguousarray(np.concatenate([cosT, cosT], axis=0))
    S2 = np.ascontiguousarray(np.concatenate([sinT, sinT], axis=0))
    S2a = np.ascontiguousarray(np.concatenate([-sinT, sinT], axis=0))
    S2b = np.ascontiguousarray(np.concatenate([sinT, -sinT], axis=0))

    if causal:
        em = np.exp(mask[0:128, 0:128].T)
    else:
        em = np.exp(mask.T)  # [k, q]
    em = np.ascontiguousarray(em.astype(ml_dtypes.bfloat16))

    ev = np.arange(0, HD, 2)
    od = np.arange(1, HD, 2)

    xb = np.ascontiguousarray(x.T).astype(ml_dtypes.bfloat16)  # [D, B*S]
    in_maps = []
    for r in range(NCORES):
        heads = [4 * r + i for i in range(HL)]
        cols = []
        for p in range(HL // 2):
            h0, h1 = heads[2 * p], heads[2 * p + 1]
            cols.append(np.concatenate([h0 * HD + ev, h1 * HD + ev]))
            cols.append(np.concatenate([h0 * HD + od, h1 * HD + od]))
        wq_r = np.ascontiguousarray(wq[:, np.concatenate(cols)])
        ka_cols = np.concatenate([r * HD + ev, r * HD + od])
        wk_r = np.ascontiguousarray(wk[:, ka_cols])
        wv_r = np.ascontiguousarray(wv[:, r * HD : (r + 1) * HD])
        wo_r = np.ascontiguousarray(wo[:, 512 * r : 512 * (r + 1)])
        m = dict(
            x=xb,
            wq_r=wq_r.astype(ml_dtypes.bfloat16),
            wk_r=wk_r.astype(ml_dtypes.bfloat16),
            wv_r=wv_r.astype(ml_dtypes.bfloat16),
            wo_r=wo_r.astype(ml_dtypes.bfloat16),
            C2=C2, S2=S2, S2a=S2a, S2b=S2b, em=em,
        )
        if not adapter_skip:
            m["adapter"] = np.ascontiguousarray(adapter.T).astype(ml_dtypes.bfloat16)
            m["tg4"] = np.ascontiguousarray(tg[4 * r : 4 * r + 4].reshape(1, 4))
        in_maps.append(m)
    return in_maps, causal, adapter_skip


def _build(causal, adapter_skip):
    nc = bacc.Bacc(trn_type="TRN2", num_devices=NCORES)

    x_d = nc.dram_tensor("x", [D, B * S], BF16, kind="ExternalInput")  # x^T
    if not adapter_skip:
        ad = nc.dram_tensor("adapter", [D, B * A_LEN], BF16, kind="ExternalInput")
    wq_r = nc.dram_tensor("wq_r", [D, 512], BF16, kind="ExternalInput")
    wk_r = nc.dram_tensor("wk_r", [D, 128], BF16, kind="ExternalInput")
    wv_r = nc.dram_tensor("wv_r", [D, 128], BF16, kind="ExternalInput")
    wo_r = nc.dram_tensor("wo_r", [D, 512], BF16, kind="ExternalInput")
    c2_d = nc.dram_tensor("C2", [128, S], F32, kind="ExternalInput")
    s2_d = nc.dram_tensor("S2", [128, S], F32, kind="ExternalInput")
    s2a_d = nc.dram_tensor("S2a", [128, S], F32, kind="ExternalInput")
    s2b_d = nc.dram_tensor("S2b", [128, S], F32, kind="ExternalInput")
    em_shape = [128, 128] if causal else [S, S]
    em_d = nc.dram_tensor("em", em_shape, BF16, kind="ExternalInput")
    if not adapter_skip:
        tg_d = nc.dram_tensor("tg4", [1, HL], F32, kind="ExternalInput")
    out_r = nc.dram_tensor("out_r", [512, B * S], F32, kind="ExternalOutput")

    rg = [list(range(NCORES))]

    with tile.TileContext(nc) as tc:
        with (
            tc.tile_pool(name="const", bufs=1) as constp,
            tc.tile_pool(name="wres", bufs=1) as wres,
            tc.tile_pool(name="xts", bufs=8) as xstream,
            tc.tile_pool(name="worhs", bufs=2 if adapter_skip else 1) as worhsp,
            tc.tile_pool(name="rtmp", bufs=1) as rtmp,
            tc.tile_pool(name="batp", bufs=2) as batp,
            tc.tile_pool(name="dram", bufs=1, space="DRAM") as dram,
        ):
            # ---- resident weights (all four, bf16, gpsimd queue) ----
            # order: group 0 of each QKV weight first so qkv0 can start,
            # then the rest; wow last (first needed ~200us in).
            wqw = wres.tile([128, 32, 512], BF16)
            kaw = wres.tile([128, 32, 128], BF16)
            wvw = wres.tile([128, 32, 128], BF16)
            wow = wres.tile([128, 32, 512], BF16)
            for g in range(8):
                gs = slice(4 * g, 4 * (g + 1))
                nc.gpsimd.dma_start(
                    kaw[:, gs, :],
                    wk_r[:].rearrange("(kt p) c -> p kt c", p=128)[:, gs, :],
                )
                nc.gpsimd.dma_start(
                    wvw[:, gs, :],
                    wv_r[:].rearrange("(kt p) c -> p kt c", p=128)[:, gs, :],
                )
                nc.gpsimd.dma_start(
                    wqw[:, gs, :],
                    wq_r[:].rearrange("(kt p) c -> p kt c", p=128)[:, gs, :],
                )
            # ---- constants (gpsimd queue: keeps sync/scalar free for xT;
            # needed first at rope time, ~50us in) ----
            ident = constp.tile([128, 128], F32)
            make_identity(nc, ident[:])
            ident_b = constp.tile([128, 128], BF16)
            nc.vector.tensor_copy(ident_b[:], ident[:])
            ones_f = constp.tile([128, 1], F32)
            nc.vector.memset(ones_f[:], 1.0)
            ones_b = constp.tile([128, 1], BF16)
            nc.vector.tensor_copy(ones_b[:], ones_f[:])
            ones1f = constp.tile([1, 128], F32)
            nc.vector.memset(ones1f[:], 1.0)
            ones1r = constp.tile([1, 128], F32R)
            nc.vector.tensor_copy(ones1r[:], ones1f[:])
            c2 = constp.tile([128, S], F32)
            nc.gpsimd.dma_start(c2[:], c2_d[:])
            s2 = constp.tile([128, S], F32)
            nc.gpsimd.dma_start(s2[:], s2_d[:])
            s2a = constp.tile([128, S], F32)
            nc.gpsimd.dma_start(s2a[:], s2a_d[:])
            s2b = constp.tile([128, S], F32)
            nc.gpsimd.dma_start(s2b[:], s2b_d[:])
            em_sb = constp.tile(em_shape if causal else [128, 4, S], BF16)
            if causal:
                nc.gpsimd.dma_start(em_sb[:], em_d[:])
            else:
                nc.gpsimd.dma_start(
                    em_sb[:], em_d[:].rearrange("(kc p) q -> p kc q", p=128)
                )
            if not adapter_skip:
                tg4 = constp.tile([1, HL], F32)
                nc.gpsimd.dma_start(tg4[:], tg_d[:])
            # wo weights last on the queue: first needed ~200us in
            for g in range(4):
                gs = slice(8 * g, 8 * (g + 1))
                nc.gpsimd.dma_start(
                    wow[:, gs, :],
                    wo_r[:].rearrange("(kt p) c -> p kt c", p=128)[:, gs, :],
                )

            # ---- adapter transpose + projections (generic path only) ----
            if not adapter_skip:
                with (
                    tc.tile_pool(name="adp", bufs=3) as adp,
                    tc.tile_pool(name="adps", bufs=2, space="PSUM") as adps,
                ):
                    akt = constp.tile([128, B, A_LEN], BF16)
                    aktb = constp.tile([128, B, A_LEN], BF16)
                    avt = adp.tile([128, B * A_LEN], BF16)
                    pk = adps.tile([128, 256], F32, name="pk")
                    pv = adps.tile([128, 256], F32, name="pv")
                    for kt in range(32):
                        adt_t = adp.tile([128, 256], BF16, tag="adt")
                        nc.sync.dma_start(
                            adt_t[:], ad[128 * kt : 128 * (kt + 1), :]
                        )
                        nc.tensor.matmul(
                            pk[:], kaw[:, kt, :], adt_t[:],
                            start=(kt == 0), stop=(kt == 31),
                        )
                        nc.tensor.matmul(
                            pv[:], wvw[:, kt, :], adt_t[:],
                            start=(kt == 0), stop=(kt == 31),
                        )
                    nc.scalar.copy(akt[:].rearrange("p b a -> p (b a)"), pk[:])
                    nc.scalar.copy(avt[:], pv[:])
                    # aKTB = swapped halves of aKT
                    nc.sync.dma_start(
                        aktb[0:64, :, :].bitcast(F32), akt[64:128, :, :].bitcast(F32)
                    )
                    nc.sync.dma_start(
                        aktb[64:128, :, :].bitcast(F32), akt[0:64, :, :].bitcast(F32)
                    )
                    # aV token-major per batch
                    av_sb = constp.tile([64, B, 128], BF16)
                    for b in range(B):
                        pav = adps.tile([64, 128], BF16)
                        nc.tensor.transpose(
                            pav[:], avt[:, 64 * b : 64 * (b + 1)], ident_b[:]
                        )
                        nc.scalar.copy(av_sb[:, b, :], pav[:])

            # ---- per-batch attnT shards in DRAM + AllGather targets ----
            at_in = []
            at_full = []
            for b in range(B):
                at_in.append(dram.tile([512, S], BF16, name=f"at_in{b}"))
                at_full.append(
                    dram.tile([D, S], BF16, addr_space="Shared", name=f"at_full{b}")
                )

            qkv_state = {}
            xt_prefetch = {}

            def prefetch_xt(b, upto):
                """Issue xt DMA loads for batch b, kt < upto (from attn blocks,
                while sync/scalar queues are idle)."""
                if b >= B:
                    return
                for kt in range(upto):
                    if (b, kt) in xt_prefetch:
                        continue
                    xt_t = xstream.tile([128, 512], BF16, tag="xt")
                    eng = nc.sync if kt % 2 == 0 else nc.scalar
                    eng.dma_start(
                        xt_t[:],
                        x_d[128 * kt : 128 * (kt + 1), 512 * b : 512 * (b + 1)],
                    )
                    xt_prefetch[(b, kt)] = xt_t

            def emit_qkv(b):
                bat = batp
                with (
                    tc.tile_pool(name=f"tps{b}", bufs=2, space="PSUM") as tps,
                    tc.tile_pool(name=f"qkvps{b}", bufs=1, space="PSUM") as qkvps,
                ):
                    q_ps = [
                        qkvps.tile([128, 512], F32, name=f"qps{m}") for m in range(4)
                    ]
                    k_ps = qkvps.tile([128, 512], F32)
                    v_ps = qkvps.tile([128, 512], F32)

                    def emit_mms(kt, xt_t):
                        st, sp = (kt == 0), (kt == 31)
                        for m in range(4):
                            nc.tensor.matmul(
                                q_ps[m][:], wqw[:, kt, 128 * m : 128 * (m + 1)],
                                xt_t[:], start=st, stop=sp,
                            )
                        nc.tensor.matmul(k_ps[:], kaw[:, kt, :], xt_t[:], start=st, stop=sp)
                        nc.tensor.matmul(v_ps[:], wvw[:, kt, :], xt_t[:], start=st, stop=sp)

                    # xT streamed by xbar DMA-transpose, 2 kt deep, on
                    # alternating sync/scalar queues.
                    pend = []
                    for kt in range(32):
                        xt_t = xt_prefetch.pop((b, kt), None)
                        if xt_t is None:
                            xt_t = xstream.tile([128, 512], BF16, tag="xt")
                            eng = nc.sync if kt % 2 == 0 else nc.scalar
                            eng.dma_start(
                                xt_t[:],
                                x_d[128 * kt : 128 * (kt + 1), 512 * b : 512 * (b + 1)],
                            )
                        pend.append((kt, xt_t))
                        if len(pend) > 2:
                            emit_mms(*pend.pop(0))
                    for p in pend:
                        emit_mms(*p)

                    # RoPE on Q pair-blocks: rqA = QA*C2 - QB*S2 ; rqB = QA*S2 + QB*C2
                    rqa, rqb = [], []
                    for p in range(2):
                        qa, qb = q_ps[2 * p], q_ps[2 * p + 1]
                        t1 = rtmp.tile([128, S], F32, tag="t1")
                        t2 = rtmp.tile([128, S], F32, tag="t2")
                        ra = bat.tile([128, S], BF16, tag=f"rqa{p}")
                        rb = bat.tile([128, S], BF16, tag=f"rqb{p}")
                        nc.vector.tensor_mul(t1[:], qa[:], c2[:])
                        nc.vector.tensor_mul(t2[:], qb[:], s2[:])
                        nc.vector.tensor_sub(ra[:], t1[:], t2[:])
                        nc.vector.tensor_mul(t1[:], qa[:], s2[:])
                        nc.vector.tensor_mul(t2[:], qb[:], c2[:])
                        nc.vector.tensor_add(rb[:], t1[:], t2[:])
                        rqa.append(ra)
                        rqb.append(rb)
                    # K: ka/kb swap-duplicate, then rope
                    ka_f = rtmp.tile([128, S], F32, tag="ka_f")
                    nc.scalar.copy(ka_f[:], k_ps[:])
                    kb_f = rtmp.tile([128, S], F32, tag="kb_f")
                    nc.scalar.dma_start(kb_f[0:64, :], ka_f[64:128, :])
                    nc.scalar.dma_start(kb_f[64:128, :], ka_f[0:64, :])
                    t1 = rtmp.tile([128, S], F32, tag="t1")
                    t2 = rtmp.tile([128, S], F32, tag="t2")
                    rka = bat.tile([128, S], BF16, tag="rka")
                    rkb = bat.tile([128, S], BF16, tag="rkb")
                    nc.vector.tensor_mul(t1[:], ka_f[:], c2[:])
                    nc.vector.tensor_mul(t2[:], kb_f[:], s2a[:])
                    nc.vector.tensor_add(rka[:], t1[:], t2[:])
                    nc.vector.tensor_mul(t1[:], kb_f[:], c2[:])
                    nc.vector.tensor_mul(t2[:], ka_f[:], s2b[:])
                    nc.vector.tensor_add(rkb[:], t1[:], t2[:])
                    # V: token-major (bf16 PE transposes)
                    vt_f = rtmp.tile([128, S], BF16, tag="vt_f")
                    nc.scalar.copy(vt_f[:], v_ps[:])
                    v_sb = bat.tile([128, 4, 128], BF16, tag="v_sb")
                    for kc in range(4):
                        pv2 = tps.tile([128, 128], BF16, tag="pst")
                        nc.tensor.transpose(
                            pv2[:], vt_f[:, 128 * kc : 128 * (kc + 1)], ident_b[:]
                        )
                        nc.scalar.copy(v_sb[:, kc, :], pv2[:])
                qkv_state[b] = (rqa, rqb, rka, rkb, v_sb)

            def emit_attn2_fast(bA, bB, pf_b=None, wo_pf=None):
                """Two-batch software-pipelined attention: every exp/recip-
                gated stage has >=2 stages (>=16 matmuls) of the other
                batch's independent work ahead of it in the PE stream, so
                the PE stays dense and the HAM clock gate stays warm."""
                states = {"A": qkv_state.pop(bA), "B": qkv_state.pop(bB)}
                bmap = {"A": bA, "B": bB}
                with (
                    tc.tile_pool(name=f"ex{bA}", bufs=2) as exp_pool,
                    tc.tile_pool(name=f"au{bA}", bufs=2) as aup,
                    tc.tile_pool(name=f"smp{bA}", bufs=1) as smp,
                    tc.tile_pool(name=f"scps{bA}", bufs=3, space="PSUM") as scps,
                    tc.tile_pool(name=f"avps{bA}", bufs=3, space="PSUM") as avps,
                    tc.tile_pool(name=f"smps{bA}", bufs=2, space="PSUM") as smps,
                ):
                    if pf_b is not None:
                        prefetch_xt(pf_b, 4)
                    if wo_pf is not None:
                        prefetch_wo_rhs(wo_pf)

                    def scores(x, pp):
                        rqa, rqb, rka, rkb, v_sb = states[x]
                        rqe, rqo = rqa[pp], rqb[pp]
                        sc_ps = {0: [], 1: []}
                        for kc in range(4):
                            qlo = 128 * kc if causal else 0
                            for hh in range(2):
                                sc_ps[hh].append(
                                    scps.tile([128, S], F32, tag="sc",
                                              name=f"sc{x}{hh}")
                                )
                            for hh, which in ((0, "e"), (1, "e"), (0, "o"), (1, "o")):
                                beta = 64 * hh
                                sl = slice(beta, beta + 64)
                                if which == "e":
                                    lh = (rka if hh == 0 else rkb)
                                    rh = rqe
                                else:
                                    lh = (rkb if hh == 0 else rka)
                                    rh = rqo
                                nc.tensor.matmul(
                                    sc_ps[hh][kc][:, qlo:S],
                                    lh[sl, 128 * kc : 128 * (kc + 1)],
                                    rh[sl, qlo:S],
                                    start=(which == "e"), stop=(which == "o"),
                                )
                        return sc_ps

                    def smav(x, pp, sc_ps):
                        v_sb = states[x][4]
                        expT = {}
                        av_p = {}
                        sm2 = smps.tile([128, S], F32, tag="sm4",
                                        name=f"sm{x}{pp}")
                        for hh in range(2):
                            expT[hh] = exp_pool.tile(
                                [128, 4, S], BF16, tag="expT", name=f"expT{hh}"
                            )
                            av_p[hh] = avps.tile(
                                [128, S], F32, tag="av", name=f"av{hh}"
                            )
                        for kc in range(4):
                            qlo = 128 * kc if causal else 0
                            for hh in range(2):
                                nc.scalar.activation(
                                    expT[hh][:, kc, qlo:S], sc_ps[hh][kc][:, qlo:S],
                                    func=mybir.ActivationFunctionType.Exp,
                                    scale=SCALE,
                                )
                                if causal:
                                    nc.vector.tensor_mul(
                                        expT[hh][:, kc, qlo : qlo + 128],
                                        expT[hh][:, kc, qlo : qlo + 128],
                                        em_sb[:],
                                    )
                                else:
                                    nc.vector.tensor_mul(
                                        expT[hh][:, kc, :],
                                        expT[hh][:, kc, :],
                                        em_sb[:, kc, :],
                                    )
                                row = 64 * hh
                                nc.tensor.matmul(
                                    sm2[row : row + 1, qlo:S], ones_b[:, 0:1],
                                    expT[hh][:, kc, qlo:S],
                                    start=(kc == 0), stop=(kc == 3),
                                )
                                nc.tensor.matmul(
                                    av_p[hh][:, qlo:S], v_sb[:, kc, :],
                                    expT[hh][:, kc, qlo:S],
                                    start=(kc == 0), stop=(kc == 3),
                                )
                        return av_p, sm2

                    def norm(x, pp, avsm):
                        av_p, sm2 = avsm
                        b = bmap[x]
                        for hh in range(2):
                            h = 2 * pp + hh
                            row = 64 * hh
                            au = aup.tile([128, S], F32, tag="attnU")
                            nc.scalar.copy(au[:], av_p[hh][:])
                            smtr = smp.tile([1, 2, S], F32, tag="smt")
                            nc.scalar.copy(smtr[:, 0, :], sm2[row : row + 1, :])
                            nc.vector.reciprocal_approx_fast(
                                smtr[:, 1, :], smtr[:, 0, :]
                            )
                            smrr = smp.tile([1, S], F32R, tag="smrr")
                            nc.vector.tensor_copy(smrr[:], smtr[:, 1, :])
                            rb_ps = avps.tile([128, S], F32, tag="av", name="rb_ps")
                            nc.tensor.matmul(
                                rb_ps[:], ones1r[0:1, :], smrr[0:1, :],
                                start=True, stop=True,
                            )
                            at_n = aup.tile([128, S], BF16, tag="at_n")
                            nc.vector.tensor_mul(at_n[:], au[:], rb_ps[:])
                            nc.gpsimd.dma_start(
                                at_in[b][128 * h : 128 * (h + 1), :], at_n[:]
                            )

                    def ag(x):
                        b = bmap[x]
                        nc.gpsimd.collective_compute(
                            "AllGather", mybir.AluOpType.bypass, replica_groups=rg,
                            ins=[at_in[b][:]], outs=[at_full[b][:]],
                        )

                    scA0 = scores("A", 0)
                    avA0 = smav("A", 0, scA0)
                    scB0 = scores("B", 0)
                    scA1 = scores("A", 1)
                    norm("A", 0, avA0)
                    avB0 = smav("B", 0, scB0)
                    avA1 = smav("A", 1, scA1)
                    scB1 = scores("B", 1)
                    norm("B", 0, avB0)
                    norm("A", 1, avA1)
                    ag("A")
                    avB1 = smav("B", 1, scB1)
                    norm("B", 1, avB1)
                    ag("B")

            def emit_attn_generic(b):
                rqa, rqb, rka, rkb, v_sb = qkv_state.pop(b)
                with (
                    tc.tile_pool(name=f"ex{b}", bufs=1) as exp_pool,
                    tc.tile_pool(name=f"au{b}", bufs=1) as aup,
                    tc.tile_pool(name=f"smp{b}", bufs=1) as smp,
                    tc.tile_pool(name=f"scps{b}", bufs=2, space="PSUM") as scps,
                    tc.tile_pool(name=f"avps{b}", bufs=2, space="PSUM") as avps,
                    tc.tile_pool(name=f"smps{b}", bufs=1, space="PSUM") as smps,
                    tc.tile_pool(name=f"ascps{b}", bufs=1, space="PSUM") as ascps,
                ):
                    for h in range(HL):
                        p, beta = h // 2, 64 * (h % 2)
                        sl = slice(beta, beta + 64)
                        rqe, rqo = rqa[p], rqb[p]
                        rke_t = rka if beta == 0 else rkb
                        rko_t = rkb if beta == 0 else rka
                        expT = exp_pool.tile([128, 4, S], BF16, tag="expT")
                        av_p = avps.tile([128, S], F32, tag="av")
                        sm_p = smps.tile([1, S], F32, tag="sm")
                        sc_ps = []
                        for kc in range(4):
                            qlo = 128 * kc if causal else 0
                            sc_p = scps.tile([128, S], F32, tag="sc")
                            sc_ps.append(sc_p)
                            nc.tensor.matmul(
                                sc_p[:, qlo:S],
                                rke_t[sl, 128 * kc : 128 * (kc + 1)],
                                rqe[sl, qlo:S],
                                start=True, stop=False,
                            )
                            nc.tensor.matmul(
                                sc_p[:, qlo:S],
                                rko_t[sl, 128 * kc : 128 * (kc + 1)],
                                rqo[sl, qlo:S],
                                start=False, stop=True,
                            )
                        for kc in range(4):
                            qlo = 128 * kc if causal else 0
                            nc.scalar.activation(
                                expT[:, kc, qlo:S], sc_ps[kc][:, qlo:S],
                                func=mybir.ActivationFunctionType.Exp, scale=SCALE,
                            )
                            if causal:
                                nc.vector.tensor_mul(
                                    expT[:, kc, qlo : qlo + 128],
                                    expT[:, kc, qlo : qlo + 128],
                                    em_sb[:],
                                )
                            else:
                                nc.vector.tensor_mul(
                                    expT[:, kc, :],
                                    expT[:, kc, :],
                                    em_sb[:, kc, :],
                                )
                            nc.tensor.matmul(
                                sm_p[0:1, qlo:S], ones_b[:, 0:1],
                                expT[:, kc, qlo:S],
                                start=(kc == 0), stop=(kc == 3),
                            )
                            nc.tensor.matmul(
                                av_p[:, qlo:S], v_sb[:, kc, :],
                                expT[:, kc, qlo:S],
                                start=(kc == 0), stop=(kc == 3),
                            )
                        au = aup.tile([128, S], F32, tag="attnU")
                        nc.scalar.copy(au[:], av_p[:])
                        smt = smp.tile([1, S], F32, tag="smt")
                        nc.scalar.copy(smt[:], sm_p[0:1, :])
                        smr = smp.tile([1, S], F32, tag="smr")
                        nc.vector.reciprocal_approx_fast(smr[:], smt[:])
                        rb_ps = avps.tile([128, S], F32, tag="av", name="rb_ps")
                        nc.tensor.matmul(
                            rb_ps[:], ones1f[0:1, :], smr[0:1, :],
                            start=True, stop=True,
                        )
                        at_n = aup.tile([128, S], BF16, tag="at_n")
                        asc_p = ascps.tile([64, S], F32, tag="asc")
                        ke_src = akt if beta == 0 else aktb
                        ko_src = aktb if beta == 0 else akt
                        nc.tensor.matmul(
                            asc_p[:], ke_src[sl, b, :], rqe[sl, :],
                            start=True, stop=False,
                        )
                        nc.tensor.matmul(
                            asc_p[:], ko_src[sl, b, :], rqo[sl, :],
                            start=False, stop=True,
                        )
                        a_expT = exp_pool.tile([64, S], BF16, tag="a_expT")
                        nc.scalar.activation(
                            a_expT[:], asc_p[:],
                            func=mybir.ActivationFunctionType.Exp, scale=SCALE,
                        )
                        asm_p = smps.tile([1, S], F32, tag="asm")
                        nc.tensor.matmul(
                            asm_p[0:1, :], ones_b[0:64, 0:1], a_expT[:],
                            start=True, stop=True,
                        )
                        aav_p = avps.tile([128, S], F32, tag="av")
                        nc.tensor.matmul(
                            aav_p[:], av_sb[:, b, :], a_expT[:],
                            start=True, stop=True,
                        )
                        aau = aup.tile([128, S], F32, tag="a_attnU")
                        nc.scalar.copy(aau[:], aav_p[:])
                        asmt = aup.tile([1, S], F32, tag="asmt")
                        nc.scalar.copy(asmt[:], asm_p[0:1, :])
                        asmr = aup.tile([1, S], F32, tag="asmr")
                        nc.vector.reciprocal_approx_fast(asmr[:], asmt[:])
                        nc.vector.tensor_scalar_mul(
                            asmr[:], asmr[:], tg4[0:1, h : h + 1]
                        )
                        arb_ps = avps.tile([128, S], F32, tag="av", name="arb_ps")
                        nc.tensor.matmul(
                            arb_ps[:], ones1f[0:1, :], asmr[0:1, :],
                            start=True, stop=True,
                        )
                        t_m = aup.tile([128, S], F32, tag="t_m")
                        nc.vector.tensor_mul(t_m[:], au[:], rb_ps[:])
                        t_a = aup.tile([128, S], F32, tag="t_a")
                        nc.vector.tensor_mul(t_a[:], aau[:], arb_ps[:])
                        nc.vector.tensor_add(at_n[:], t_m[:], t_a[:])
                        nc.gpsimd.dma_start(
                            at_in[b][128 * h : 128 * (h + 1), :], at_n[:]
                        )

                nc.gpsimd.collective_compute(
                    "AllGather", mybir.AluOpType.bypass, replica_groups=rg,
                    ins=[at_in[b][:]], outs=[at_full[b][:]],
                )

            wo_rhs_cache = {}

            def prefetch_wo_rhs(b):
                if b in wo_rhs_cache:
                    return
                rhs_t = worhsp.tile(
                    [128, 16, 2, 512], BF16, tag="worhs", name=f"worhs{b}"
                )
                for kp in range(16):
                    eng = nc.sync if kp % 2 == 0 else nc.scalar
                    eng.dma_start(
                        rhs_t[:, kp, :, :],
                        at_full[b][
                            256 * kp : 256 * (kp + 1), :
                        ].rearrange("(two p) t -> p two t", p=128),
                    )
                wo_rhs_cache[b] = rhs_t

            def make_wo_chunks(b, woop, wops):
                """wo GEMM for batch b as a list of chunk closures: rhs
                resident up-front (double-buffered across batches), then
                per-m 32-kt accumulation in a single PSUM bank (double-
                buffered across m), evacuated per m."""
                prefetch_wo_rhs(b)
                rhs_t = wo_rhs_cache.pop(b)
                state = {}

                def mk_mm(m, kp_lo, kp_hi):
                    def go():
                        if m not in state:
                            state[m] = wops.tile(
                                [128, 512], F32, tag="ops", name=f"ops{m}_{b}"
                            )
                        o_ps = state[m]
                        for kp in range(kp_lo, kp_hi):
                            for j in range(2):
                                kt = 2 * kp + j
                                nc.tensor.matmul(
                                    o_ps[:],
                                    wow[:, kt, 128 * m : 128 * (m + 1)],
                                    rhs_t[:, kp, j, :],
                                    start=(kt == 0), stop=(kt == 31),
                                )
                    return go

                def mk_evac(m):
                    def go():
                        o_ps = state.pop(m)
                        osb = woop.tile([128, 512], F32, tag="osb")
                        nc.scalar.copy(osb[:], o_ps[:])
                        nc.sync.dma_start(
                            out_r[
                                128 * m : 128 * (m + 1),
                                512 * b : 512 * (b + 1),
                            ],
                            osb[:],
                        )
                    return go

                chunks = []
                for m in range(4):
                    for kp_lo in range(0, 16, 4):
                        chunks.append(mk_mm(m, kp_lo, kp_lo + 4))
                    chunks.append(mk_evac(m))
                return chunks

            def emit_wo(b):
                with (
                    tc.tile_pool(name=f"woo{b}w", bufs=2) as woop,
                    tc.tile_pool(name=f"wops{b}w", bufs=2, space="PSUM") as wops,
                ):
                    for ch in make_wo_chunks(b, woop, wops):
                        ch()

            if adapter_skip:
                emit_qkv(0)
                emit_qkv(1)
                emit_attn2_fast(0, 1, pf_b=2)
                emit_qkv(2)
                emit_qkv(3)
                emit_attn2_fast(2, 3, wo_pf=0)
                emit_wo(0)
                emit_wo(1)
                emit_wo(2)
                emit_wo(3)
            else:
                emit_qkv(0)
                emit_qkv(1)
                emit_attn_generic(0)
                emit_qkv(2)
                emit_attn_generic(1)
                emit_qkv(3)
                emit_attn_generic(2)
                emit_wo(0)
                emit_attn_generic(3)
                emit_wo(1)
                emit_wo(2)
                emit_wo(3)

    nc.compile()
    return nc


def kernel(**inputs) -> np.ndarray:
    in_maps, causal, adapter_skip = _host_prep(inputs)
    key = (causal, adapter_skip)
    if key not in _cache:
        _cache[key] = _build(causal, adapter_skip)
    nc = _cache[key]
    res = run_bass_kernel_spmd(nc, in_maps, core_ids=list(range(NCORES)))
    global last_result
    last_result = res
    out = np.empty((B * S, D), np.float32)
    for r in range(NCORES):
        out[:, 512 * r : 512 * (r + 1)] = res.results[r]["out_r"].T
    return out.reshape(B, S, D)


if __name__ == "__main__":
    rng = np.random.default_rng(0)
    demo = {
        "x": rng.standard_normal((B, S, D), dtype=np.float32),
        "adapter": rng.standard_normal((B, A_LEN, D), dtype=np.float32),
        "mask": np.where(
            np.tril(np.ones((S, S), dtype=bool)), 0.0, -1e9
        ).astype(np.float32)[None, None],
        "freqs_cos": rng.random((S, 64), dtype=np.float32),
        "freqs_sin": rng.random((S, 64), dtype=np.float32),
        "wq": (rng.standard_normal((D, H * HD), dtype=np.float32) * 0.02),
        "wk": (rng.standard_normal((D, HK * HD), dtype=np.float32) * 0.02),
        "wv": (rng.standard_normal((D, HK * HD), dtype=np.float32) * 0.02),
        "wo": (rng.standard_normal((H * HD, D), dtype=np.float32) * 0.02),
        "gate": np.zeros((1, H, 1, 1), np.float32),
    }
    o = kernel(**demo)
    print("kernel ran, out shape", o.shape)


# revision 30
# speedup vs baseline: 1.0660x; 1.0170x over previous
"""Trainium2 Bass kernel for nn_Attention_335007449334 (8-core TP attention).

Strategy: tensor-parallel over heads across 8 NeuronCores (SPMD, one program).
  - Each core owns 4 query heads + 1 kv head: wq/wk/wv column-sharded on host.
  - All matmuls run in bf16 (inputs rounded host-side / on-engine; PSUM
    accumulation stays fp32): 1 cyc/row on the PE vs 1.5 for f32r, and x is
    transposed for free by xbar DMA-transpose (2-byte dtype requirement).
  - All four weight sets (wq/wk/wv/wo) are SBUF-resident from the start
    (bf16 halves their footprint) so no mid-stream weight swap is needed.
  - Attention is computed fully transposed (scoresT [k, q]) so no
    probs-transpose is needed: softmax sums come from ones-matmuls (max-
    subtraction is skipped; score range is tiny), the causal mask is applied
    multiplicatively post-exp on the diagonal block only (off-diagonal blocks
    use restricted matmul N ranges), and normalization folds into the
    PSUM->SBUF evacuation via K=1 broadcast matmuls of the row reciprocals
    (kept f32r for precision).
  - RoPE runs on an even/odd head-dim permutation baked into the host-side
    weight column order; the K head is duplicated into swapped-half tiles so
    every DVE op is base-partition aligned. Head pairs are interleaved so
    their K=64 score matmuls pack into disjoint PE row groups.
  - The adapter cross-attention path is emitted only when tanh(gate) != 0
    (it is exactly zero otherwise); the causal fast path is used only when
    the mask matches the canonical causal pattern.
  - Per-batch attnT shards are AllGathered in bf16 (overlapped with later
    batches); wo is column-sharded and emitted per batch right after the
    batch's AllGather has had time to complete, so the final AllGather hides
    behind the previous batch's wo GEMM. Each core emits
    out^T[:, 512r:512r+512]; the host concatenates + transposes.
"""

import sys
import numpy as np
import ml_dtypes

sys.path.insert(0, "/opt/trn_rl_repo")

import concourse.bass as bass  # noqa: E402
import concourse.tile as tile  # noqa: E402
from concourse import bacc, mybir  # noqa: E402
from concourse.bass_utils import run_bass_kernel_spmd  # noqa: E402
from concourse.masks import make_identity  # noqa: E402

# If BASS_TRACE is set but this image lacks antenv.axon_hooks, bass_utils
# would crash on import; provide a stub so tracing degrades gracefully.
try:  # noqa: SIM105
    import antenv.axon_hooks  # noqa: F401
except ImportError:
    import types as _types

    try:
        import antenv  # noqa: F401

        _hooks = _types.ModuleType("antenv.axon_hooks")
        _hh = {"hook": None}
        _hooks.set_axon_ntff_profile_hook = lambda h: _hh.__setitem__("hook", h)
        _hooks.get_axon_ntff_profile_hook = lambda: _hh["hook"]
        sys.modules["antenv.axon_hooks"] = _hooks
    except ImportError:
        pass

B, S, D = 4, 512, 4096
H, HK, HD = 32, 8, 128
NCORES = 8
HL = H // NCORES  # 4 local q-heads per core
A_LEN = 64
SCALE = 1.0 / float(np.sqrt(HD))

F32 = mybir.dt.float32
F32R = mybir.dt.float32r
BF16 = mybir.dt.bfloat16

_cache = {}
last_result = None


def _host_prep(inputs):
    x = np.asarray(inputs["x"], np.float32).reshape(B * S, D)
    adapter = np.asarray(inputs["adapter"], np.float32).reshape(B * A_LEN, D)
    mask = np.asarray(inputs["mask"], np.float32)[0, 0]
    cos = np.asarray(inputs["freqs_cos"], np.float32)
    sin = np.asarray(inputs["freqs_sin"], np.float32)
    wq = np.asarray(inputs["wq"], np.float32)
    wk = np.asarray(inputs["wk"], np.float32)
    wv = np.asarray(inputs["wv"], np.float32)
    wo = np.asarray(inputs["wo"], np.float32)
    gate = np.asarray(inputs["gate"], np.float32)[0, :, 0, 0]
    tg = np.tanh(gate).astype(np.float32)

    canonical = np.where(
        np.tril(np.ones((S, S), dtype=bool)), np.float32(0.0), np.float32(-1e9)
    ).astype(np.float32)
    causal = bool(np.array_equal(mask, canonical))
    adapter_skip = bool(np.all(tg == 0.0))

    cosT = np.ascontiguousarray(cos.T)  # [64, S]
    sinT = np.ascontiguousarray(sin.T)
    C2 = np.ascontiguousarray(np.concatenate([cosT, cosT], axis=0))
    S2a = np.ascontiguousarray(np.concatenate([-sinT, sinT], axis=0))

    if causal:
        em = np.exp(mask[0:128, 0:128].T)
    else:
        em = np.exp(mask.T)  # [k, q]
    em = np.ascontiguousarray(em.astype(ml_dtypes.bfloat16))

    ev = np.arange(0, HD, 2)
    od = np.arange(1, HD, 2)

    xb = np.ascontiguousarray(x.T).astype(ml_dtypes.bfloat16)  # [D, B*S]
    in_maps = []
    for r in range(NCORES):
        heads = [4 * r + i for i in range(HL)]
        cols = [np.concatenate([h * HD + ev, h * HD + od]) for h in heads]
        wq_r = np.ascontiguousarray(wq[:, np.concatenate(cols)])
        ka_cols = np.concatenate([r * HD + ev, r * HD + od])
        wk_r = np.ascontiguousarray(wk[:, ka_cols])
        wv_r = np.ascontiguousarray(wv[:, r * HD : (r + 1) * HD])
        wo_r = np.ascontiguousarray(wo[:, 512 * r : 512 * (r + 1)])
        m = dict(
            x=xb,
            wq_r=wq_r.astype(ml_dtypes.bfloat16),
            wk_r=wk_r.astype(ml_dtypes.bfloat16),
            wv_r=wv_r.astype(ml_dtypes.bfloat16),
            wo_r=wo_r.astype(ml_dtypes.bfloat16),
            C2=C2, S2a=S2a, em=em,
        )
        if not adapter_skip:
            m["adapter"] = np.ascontiguousarray(adapter.T).astype(ml_dtypes.bfloat16)
            m["tg4"] = np.ascontiguousarray(tg[4 * r : 4 * r + 4].reshape(1, 4))
        in_maps.append(m)
    return in_maps, causal, adapter_skip


def _build(causal, adapter_skip):
    nc = bacc.Bacc(trn_type="TRN2", num_devices=NCORES)

    x_d = nc.dram_tensor("x", [D, B * S], BF16, kind="ExternalInput")  # x^T
    if not adapter_skip:
        ad = nc.dram_tensor("adapter", [D, B * A_LEN], BF16, kind="ExternalInput")
    wq_r = nc.dram_tensor("wq_r", [D, 512], BF16, kind="ExternalInput")
    wk_r = nc.dram_tensor("wk_r", [D, 128], BF16, kind="ExternalInput")
    wv_r = nc.dram_tensor("wv_r", [D, 128], BF16, kind="ExternalInput")
    wo_r = nc.dram_tensor("wo_r", [D, 512], BF16, kind="ExternalInput")
    c2_d = nc.dram_tensor("C2", [128, S], F32, kind="ExternalInput")
    s2a_d = nc.dram_tensor("S2a", [128, S], F32, kind="ExternalInput")
    em_shape = [128, 128] if causal else [S, S]
    em_d = nc.dram_tensor("em", em_shape, BF16, kind="ExternalInput")
    if not adapter_skip:
        tg_d = nc.dram_tensor("tg4", [1, HL], F32, kind="ExternalInput")
    out_r = nc.dram_tensor("out_r", [512, B * S], F32, kind="ExternalOutput")

    rg = [list(range(NCORES))]

    with tile.TileContext(nc) as tc:
        with (
            tc.tile_pool(name="const", bufs=1) as constp,
            tc.tile_pool(name="wres", bufs=1) as wres,
            tc.tile_pool(name="xts", bufs=8) as xstream,
            tc.tile_pool(name="worhs", bufs=2 if adapter_skip else 1) as worhsp,
            tc.tile_pool(name="rtmp", bufs=1) as rtmp,
            tc.tile_pool(name="batp", bufs=2) as batp,
            tc.tile_pool(name="dram", bufs=1, space="DRAM") as dram,
        ):
            # ---- resident weights (all four, bf16, gpsimd queue) ----
            # order: group 0 of each QKV weight first so qkv0 can start,
            # then the rest; wow last (first needed ~200us in).
            wqw = wres.tile([128, 32, 512], BF16)
            kaw = wres.tile([128, 32, 128], BF16)
            wvw = wres.tile([128, 32, 128], BF16)
            wow = wres.tile([128, 32, 512], BF16)
            for g in range(8):
                gs = slice(4 * g, 4 * (g + 1))
                nc.gpsimd.dma_start(
                    kaw[:, gs, :],
                    wk_r[:].rearrange("(kt p) c -> p kt c", p=128)[:, gs, :],
                )
                nc.gpsimd.dma_start(
                    wvw[:, gs, :],
                    wv_r[:].rearrange("(kt p) c -> p kt c", p=128)[:, gs, :],
                )
                nc.gpsimd.dma_start(
                    wqw[:, gs, :],
                    wq_r[:].rearrange("(kt p) c -> p kt c", p=128)[:, gs, :],
                )
            # ---- constants (gpsimd queue: keeps sync/scalar free for xT;
            # needed first at rope time, ~50us in) ----
            ident = constp.tile([128, 128], F32)
            make_identity(nc, ident[:])
            ident_b = constp.tile([128, 128], BF16)
            nc.vector.tensor_copy(ident_b[:], ident[:])
            ones_f = constp.tile([128, 1], F32)
            nc.vector.memset(ones_f[:], 1.0)
            ones_b = constp.tile([128, 1], BF16)
            nc.vector.tensor_copy(ones_b[:], ones_f[:])
            ones1f = constp.tile([1, 128], F32)
            nc.vector.memset(ones1f[:], 1.0)
            ones1r = constp.tile([1, 128], F32R)
            nc.vector.tensor_copy(ones1r[:], ones1f[:])
            c2 = constp.tile([128, S], F32)
            nc.gpsimd.dma_start(c2[:], c2_d[:])
            s2a = constp.tile([128, S], F32)
            nc.gpsimd.dma_start(s2a[:], s2a_d[:])
            em_sb = constp.tile(em_shape if causal else [128, 4, S], BF16)
            if causal:
                nc.gpsimd.dma_start(em_sb[:], em_d[:])
            else:
                nc.gpsimd.dma_start(
                    em_sb[:], em_d[:].rearrange("(kc p) q -> p kc q", p=128)
                )
            if not adapter_skip:
                tg4 = constp.tile([1, HL], F32)
                nc.gpsimd.dma_start(tg4[:], tg_d[:])
            # wo weights last on the queue: first needed ~200us in
            for g in range(4):
                gs = slice(8 * g, 8 * (g + 1))
                nc.gpsimd.dma_start(
                    wow[:, gs, :],
                    wo_r[:].rearrange("(kt p) c -> p kt c", p=128)[:, gs, :],
                )

            # ---- adapter transpose + projections (generic path only) ----
            if not adapter_skip:
                with (
                    tc.tile_pool(name="adp", bufs=3) as adp,
                    tc.tile_pool(name="adps", bufs=2, space="PSUM") as adps,
                ):
                    akt = constp.tile([128, B, A_LEN], BF16)
                    avt = adp.tile([128, B * A_LEN], BF16)
                    pk = adps.tile([128, 256], F32, name="pk")
                    pv = adps.tile([128, 256], F32, name="pv")
                    for kt in range(32):
                        adt_t = adp.tile([128, 256], BF16, tag="adt")
                        nc.sync.dma_start(
                            adt_t[:], ad[128 * kt : 128 * (kt + 1), :]
                        )
                        nc.tensor.matmul(
                            pk[:], kaw[:, kt, :], adt_t[:],
                            start=(kt == 0), stop=(kt == 31),
                        )
                        nc.tensor.matmul(
                            pv[:], wvw[:, kt, :], adt_t[:],
                            start=(kt == 0), stop=(kt == 31),
                        )
                    nc.scalar.copy(akt[:].rearrange("p b a -> p (b a)"), pk[:])
                    nc.scalar.copy(avt[:], pv[:])
                    # aV token-major per batch
                    av_sb = constp.tile([64, B, 128], BF16)
                    for b in range(B):
                        pav = adps.tile([64, 128], BF16)
                        nc.tensor.transpose(
                            pav[:], avt[:, 64 * b : 64 * (b + 1)], ident_b[:]
                        )
                        nc.scalar.copy(av_sb[:, b, :], pav[:])

            # ---- per-batch attnT shards in DRAM + AllGather targets ----
            at_in = []
            at_full = []
            for b in range(B):
                at_in.append(dram.tile([512, S], BF16, name=f"at_in{b}"))
                at_full.append(
                    dram.tile([D, S], BF16, addr_space="Shared", name=f"at_full{b}")
                )

            qkv_state = {}
            xt_prefetch = {}

            def prefetch_xt(b, upto):
                """Issue xt DMA loads for batch b, kt < upto (from attn blocks,
                while sync/scalar queues are idle)."""
                if b >= B:
                    return
                for kt in range(upto):
                    if (b, kt) in xt_prefetch:
                        continue
                    xt_t = xstream.tile([128, 512], BF16, tag="xt")
                    eng = nc.sync if kt % 2 == 0 else nc.scalar
                    eng.dma_start(
                        xt_t[:],
                        x_d[128 * kt : 128 * (kt + 1), 512 * b : 512 * (b + 1)],
                    )
                    xt_prefetch[(b, kt)] = xt_t

            def emit_qkv(b):
                bat = batp
                with (
                    tc.tile_pool(name=f"tps{b}", bufs=2, space="PSUM") as tps,
                    tc.tile_pool(name=f"qkvps{b}", bufs=1, space="PSUM") as qkvps,
                ):
                    q_ps = [
                        qkvps.tile([128, 512], F32, name=f"qps{m}") for m in range(4)
                    ]
                    k_ps = qkvps.tile([128, 512], F32)
                    v_ps = qkvps.tile([128, 512], F32)

                    def emit_mms(kt, xt_t):
                        st, sp = (kt == 0), (kt == 31)
                        for m in range(4):
                            nc.tensor.matmul(
                                q_ps[m][:], wqw[:, kt, 128 * m : 128 * (m + 1)],
                                xt_t[:], start=st, stop=sp,
                            )
                        nc.tensor.matmul(k_ps[:], kaw[:, kt, :], xt_t[:], start=st, stop=sp)
                        nc.tensor.matmul(v_ps[:], wvw[:, kt, :], xt_t[:], start=st, stop=sp)

                    # xT streamed by xbar DMA-transpose, 2 kt deep, on
                    # alternating sync/scalar queues.
                    pend = []
                    for kt in range(32):
                        xt_t = xt_prefetch.pop((b, kt), None)
                        if xt_t is None:
                            xt_t = xstream.tile([128, 512], BF16, tag="xt")
                            eng = nc.sync if kt % 2 == 0 else nc.scalar
                            eng.dma_start(
                                xt_t[:],
                                x_d[128 * kt : 128 * (kt + 1), 512 * b : 512 * (b + 1)],
                            )
                        pend.append((kt, xt_t))
                        if len(pend) > 2:
                            emit_mms(*pend.pop(0))
                    for p in pend:
                        emit_mms(*p)

                    # RoPE, stacked [e; o] per-head layout: both the e' and
                    # o' halves come out of one formula r = v*C2 + v_sw*S2a
                    # where v_sw is the half-swapped copy.
                    rq = []
                    for h in range(HL):
                        qf = rtmp.tile([128, S], F32, tag="qf", bufs=2)
                        nc.scalar.copy(qf[:], q_ps[h][:])
                        qsw = rtmp.tile([128, S], F32, tag="qsw", bufs=2)
                        eng = nc.sync if h % 2 == 0 else nc.scalar
                        eng.dma_start(qsw[0:64, :], qf[64:128, :])
                        eng.dma_start(qsw[64:128, :], qf[0:64, :])
                        t1 = rtmp.tile([128, S], F32, tag="t1")
                        t2 = rtmp.tile([128, S], F32, tag="t2")
                        rq_h = bat.tile([128, S], BF16, tag=f"rq{h}")
                        nc.vector.tensor_mul(t1[:], qf[:], c2[:])
                        nc.vector.tensor_mul(t2[:], qsw[:], s2a[:])
                        nc.vector.tensor_add(rq_h[:], t1[:], t2[:])
                        rq.append(rq_h)
                    # K: same stacked rope
                    ka_f = rtmp.tile([128, S], F32, tag="ka_f")
                    nc.scalar.copy(ka_f[:], k_ps[:])
                    kb_f = rtmp.tile([128, S], F32, tag="kb_f")
                    nc.scalar.dma_start(kb_f[0:64, :], ka_f[64:128, :])
                    nc.scalar.dma_start(kb_f[64:128, :], ka_f[0:64, :])
                    t1 = rtmp.tile([128, S], F32, tag="t1")
                    t2 = rtmp.tile([128, S], F32, tag="t2")
                    rka = bat.tile([128, S], BF16, tag="rka")
                    nc.vector.tensor_mul(t1[:], ka_f[:], c2[:])
                    nc.vector.tensor_mul(t2[:], kb_f[:], s2a[:])
                    nc.vector.tensor_add(rka[:], t1[:], t2[:])
                    # V: token-major (bf16 PE transposes)
                    vt_f = rtmp.tile([128, S], BF16, tag="vt_f")
                    nc.scalar.copy(vt_f[:], v_ps[:])
                    v_sb = bat.tile([128, 4, 128], BF16, tag="v_sb")
                    for kc in range(4):
                        pv2 = tps.tile([128, 128], BF16, tag="pst")
                        nc.tensor.transpose(
                            pv2[:], vt_f[:, 128 * kc : 128 * (kc + 1)], ident_b[:]
                        )
                        nc.scalar.copy(v_sb[:, kc, :], pv2[:])
                qkv_state[b] = (rq, rka, v_sb)

            def emit_attn2_fast(bA, bB, pf_b=None, wo_pf=None):
                """Two-batch software-pipelined attention: every exp/recip-
                gated stage has >=2 stages (>=16 matmuls) of the other
                batch's independent work ahead of it in the PE stream, so
                the PE stays dense and the HAM clock gate stays warm."""
                states = {"A": qkv_state.pop(bA), "B": qkv_state.pop(bB)}  # (rq, rka, v_sb)
                bmap = {"A": bA, "B": bB}
                with (
                    tc.tile_pool(name=f"ex{bA}", bufs=2) as exp_pool,
                    tc.tile_pool(name=f"au{bA}", bufs=2) as aup,
                    tc.tile_pool(name=f"smp{bA}", bufs=1) as smp,
                    tc.tile_pool(name=f"scps{bA}", bufs=3, space="PSUM") as scps,
                    tc.tile_pool(name=f"avps{bA}", bufs=3, space="PSUM") as avps,
                    tc.tile_pool(name=f"smps{bA}", bufs=2, space="PSUM") as smps,
                ):
                    if pf_b is not None:
                        prefetch_xt(pf_b, 4)
                    if wo_pf is not None:
                        prefetch_wo_rhs(wo_pf)

                    def scores(x, pp):
                        rq, rka, v_sb = states[x]
                        sc_ps = {0: [], 1: []}
                        for kc in range(4):
                            qlo = 128 * kc if causal else 0
                            for hh in range(2):
                                sc = scps.tile([128, S], F32, tag="sc",
                                               name=f"sc{x}{hh}")
                                sc_ps[hh].append(sc)
                                nc.tensor.matmul(
                                    sc[:, qlo:S],
                                    rka[:, 128 * kc : 128 * (kc + 1)],
                                    rq[2 * pp + hh][:, qlo:S],
                                    start=True, stop=True,
                                )
                        return sc_ps

                    def smav(x, pp, sc_ps):
                        v_sb = states[x][2]
                        expT = {}
                        av_p = {}
                        sm2 = smps.tile([128, S], F32, tag="sm4",
                                        name=f"sm{x}{pp}")
                        for hh in range(2):
                            expT[hh] = exp_pool.tile(
                                [128, 4, S], BF16, tag="expT", name=f"expT{hh}"
                            )
                            av_p[hh] = avps.tile(
                                [128, S], F32, tag="av", name=f"av{hh}"
                            )
                        for kc in range(4):
                            qlo = 128 * kc if causal else 0
                            for hh in range(2):
                                nc.scalar.activation(
                                    expT[hh][:, kc, qlo:S], sc_ps[hh][kc][:, qlo:S],
                                    func=mybir.ActivationFunctionType.Exp,
                                    scale=SCALE,
                                )
                                if causal:
                                    nc.vector.tensor_mul(
                                        expT[hh][:, kc, qlo : qlo + 128],
                                        expT[hh][:, kc, qlo : qlo + 128],
                                        em_sb[:],
                                    )
                                else:
                                    nc.vector.tensor_mul(
                                        expT[hh][:, kc, :],
                                        expT[hh][:, kc, :],
                                        em_sb[:, kc, :],
                                    )
                                row = 64 * hh
                                nc.tensor.matmul(
                                    sm2[row : row + 1, qlo:S], ones_b[:, 0:1],
                                    expT[hh][:, kc, qlo:S],
                                    start=(kc == 0), stop=(kc == 3),
                                )
                                nc.tensor.matmul(
                                    av_p[hh][:, qlo:S], v_sb[:, kc, :],
                                    expT[hh][:, kc, qlo:S],
                                    start=(kc == 0), stop=(kc == 3),
                                )
                        return av_p, sm2

                    def norm(x, pp, avsm):
                        av_p, sm2 = avsm
                        b = bmap[x]
                        for hh in range(2):
                            h = 2 * pp + hh
                            row = 64 * hh
                            au = aup.tile([128, S], F32, tag="attnU")
                            nc.scalar.copy(au[:], av_p[hh][:])
                            smtr = smp.tile([1, 2, S], F32, tag="smt")
                            nc.scalar.copy(smtr[:, 0, :], sm2[row : row + 1, :])
                            nc.vector.reciprocal_approx_fast(
                                smtr[:, 1, :], smtr[:, 0, :]
                            )
                            smrr = smp.tile([1, S], F32R, tag="smrr")
                            nc.vector.tensor_copy(smrr[:], smtr[:, 1, :])
                            rb_ps = avps.tile([128, S], F32, tag="av", name="rb_ps")
                            nc.tensor.matmul(
                                rb_ps[:], ones1r[0:1, :], smrr[0:1, :],
                                start=True, stop=True,
                            )
                            at_n = aup.tile([128, S], BF16, tag="at_n")
                            nc.vector.tensor_mul(at_n[:], au[:], rb_ps[:])
                            eng = nc.sync if hh == 0 else nc.scalar
                            eng.dma_start(
                                at_in[b][128 * h : 128 * (h + 1), :], at_n[:]
                            )

                    def ag(x):
                        b = bmap[x]
                        nc.gpsimd.collective_compute(
                            "AllGather", mybir.AluOpType.bypass, replica_groups=rg,
                            ins=[at_in[b][:]], outs=[at_full[b][:]],
                        )

                    scA0 = scores("A", 0)
                    avA0 = smav("A", 0, scA0)
                    scB0 = scores("B", 0)
                    scA1 = scores("A", 1)
                    norm("A", 0, avA0)
                    avB0 = smav("B", 0, scB0)
                    avA1 = smav("A", 1, scA1)
                    scB1 = scores("B", 1)
                    norm("B", 0, avB0)
                    norm("A", 1, avA1)
                    ag("A")
                    avB1 = smav("B", 1, scB1)
                    norm("B", 1, avB1)
                    ag("B")

            def emit_attn_generic(b):
                rq, rka, v_sb = qkv_state.pop(b)
                with (
                    tc.tile_pool(name=f"ex{b}", bufs=1) as exp_pool,
                    tc.tile_pool(name=f"au{b}", bufs=1) as aup,
                    tc.tile_pool(name=f"smp{b}", bufs=1) as smp,
                    tc.tile_pool(name=f"scps{b}", bufs=2, space="PSUM") as scps,
                    tc.tile_pool(name=f"avps{b}", bufs=2, space="PSUM") as avps,
                    tc.tile_pool(name=f"smps{b}", bufs=1, space="PSUM") as smps,
                    tc.tile_pool(name=f"ascps{b}", bufs=1, space="PSUM") as ascps,
                ):
                    for h in range(HL):
                        rq_h = rq[h]
                        expT = exp_pool.tile([128, 4, S], BF16, tag="expT")
                        av_p = avps.tile([128, S], F32, tag="av")
                        sm_p = smps.tile([1, S], F32, tag="sm")
                        sc_ps = []
                        for kc in range(4):
                            qlo = 128 * kc if causal else 0
                            sc_p = scps.tile([128, S], F32, tag="sc")
                            sc_ps.append(sc_p)
                            nc.tensor.matmul(
                                sc_p[:, qlo:S],
                                rka[:, 128 * kc : 128 * (kc + 1)],
                                rq_h[:, qlo:S],
                                start=True, stop=True,
                            )
                        for kc in range(4):
                            qlo = 128 * kc if causal else 0
                            nc.scalar.activation(
                                expT[:, kc, qlo:S], sc_ps[kc][:, qlo:S],
                                func=mybir.ActivationFunctionType.Exp, scale=SCALE,
                            )
                            if causal:
                                nc.vector.tensor_mul(
                                    expT[:, kc, qlo : qlo + 128],
                                    expT[:, kc, qlo : qlo + 128],
                                    em_sb[:],
                                )
                            else:
                                nc.vector.tensor_mul(
                                    expT[:, kc, :],
                                    expT[:, kc, :],
                                    em_sb[:, kc, :],
                                )
                            nc.tensor.matmul(
                                sm_p[0:1, qlo:S], ones_b[:, 0:1],
                                expT[:, kc, qlo:S],
                                start=(kc == 0), stop=(kc == 3),
                            )
                            nc.tensor.matmul(
                                av_p[:, qlo:S], v_sb[:, kc, :],
                                expT[:, kc, qlo:S],
                                start=(kc == 0), stop=(kc == 3),
                            )
                        au = aup.tile([128, S], F32, tag="attnU")
                        nc.scalar.copy(au[:], av_p[:])
                        smt = smp.tile([1, S], F32, tag="smt")
                        nc.scalar.copy(smt[:], sm_p[0:1, :])
                        smr = smp.tile([1, S], F32, tag="smr")
                        nc.vector.reciprocal_approx_fast(smr[:], smt[:])
                        rb_ps = avps.tile([128, S], F32, tag="av", name="rb_ps")
                        nc.tensor.matmul(
                            rb_ps[:], ones1f[0:1, :], smr[0:1, :],
                            start=True, stop=True,
                        )
                        at_n = aup.tile([128, S], BF16, tag="at_n")
                        asc_p = ascps.tile([64, S], F32, tag="asc")
                        nc.tensor.matmul(
                            asc_p[:], akt[:, b, :], rq_h[:],
                            start=True, stop=True,
                        )
                        a_expT = exp_pool.tile([64, S], BF16, tag="a_expT")
                        nc.scalar.activation(
                            a_expT[:], asc_p[:],
                            func=mybir.ActivationFunctionType.Exp, scale=SCALE,
                        )
                        asm_p = smps.tile([1, S], F32, tag="asm")
                        nc.tensor.matmul(
                            asm_p[0:1, :], ones_b[0:64, 0:1], a_expT[:],
                            start=True, stop=True,
                        )
                        aav_p = avps.tile([128, S], F32, tag="av")
                        nc.tensor.matmul(
                            aav_p[:], av_sb[:, b, :], a_expT[:],
                            start=True, stop=True,
                        )
                        aau = aup.tile([128, S], F32, tag="a_attnU")
                        nc.scalar.copy(aau[:], aav_p[:])
                        asmt = aup.tile([1, S], F32, tag="asmt")
                        nc.scalar.copy(asmt[:], asm_p[0:1, :])
                        asmr = aup.tile([1, S], F32, tag="asmr")
                        nc.vector.reciprocal_approx_fast(asmr[:], asmt[:])
                        nc.vector.tensor_scalar_mul(
                            asmr[:], asmr[:], tg4[0:1, h : h + 1]
                        )
                        arb_ps = avps.tile([128, S], F32, tag="av", name="arb_ps")
                        nc.tensor.matmul(
                            arb_ps[:], ones1f[0:1, :], asmr[0:1, :],
                            start=True, stop=True,
                        )
                        t_m = aup.tile([128, S], F32, tag="t_m")
                        nc.vector.tensor_mul(t_m[:], au[:], rb_ps[:])
                        t_a = aup.tile([128, S], F32, tag="t_a")
                        nc.vector.tensor_mul(t_a[:], aau[:], arb_ps[:])
                        nc.vector.tensor_add(at_n[:], t_m[:], t_a[:])
                        nc.gpsimd.dma_start(
                            at_in[b][128 * h : 128 * (h + 1), :], at_n[:]
                        )

                nc.gpsimd.collective_compute(
                    "AllGather", mybir.AluOpType.bypass, replica_groups=rg,
                    ins=[at_in[b][:]], outs=[at_full[b][:]],
                )

            wo_rhs_cache = {}

            def prefetch_wo_rhs(b):
                if b in wo_rhs_cache:
                    return
                rhs_t = worhsp.tile(
                    [128, 16, 2, 512], BF16, tag="worhs", name=f"worhs{b}"
                )
                for kp in range(16):
                    eng = nc.sync if kp % 2 == 0 else nc.scalar
                    eng.dma_start(
                        rhs_t[:, kp, :, :],
                        at_full[b][
                            256 * kp : 256 * (kp + 1), :
                        ].rearrange("(two p) t -> p two t", p=128),
                    )
                wo_rhs_cache[b] = rhs_t

            def make_wo_chunks(b, woop, wops):
                """wo GEMM for batch b as a list of chunk closures: rhs
                resident up-front (double-buffered across batches), then
                per-m 32-kt accumulation in a single PSUM bank (double-
                buffered across m), evacuated per m."""
                prefetch_wo_rhs(b)
                rhs_t = wo_rhs_cache.pop(b)
                state = {}

                def mk_mm(m, kp_lo, kp_hi):
                    def go():
                        if m not in state:
                            state[m] = wops.tile(
                                [128, 512], F32, tag="ops", name=f"ops{m}_{b}"
                            )
                        o_ps = state[m]
                        for kp in range(kp_lo, kp_hi):
                            for j in range(2):
                                kt = 2 * kp + j
                                nc.tensor.matmul(
                                    o_ps[:],
                                    wow[:, kt, 128 * m : 128 * (m + 1)],
                                    rhs_t[:, kp, j, :],
                                    start=(kt == 0), stop=(kt == 31),
                                )
                    return go

                def mk_evac(m):
                    def go():
                        o_ps = state.pop(m)
                        osb = woop.tile([128, 512], F32, tag="osb")
                        nc.scalar.copy(osb[:], o_ps[:])
                        nc.sync.dma_start(
                            out_r[
                                128 * m : 128 * (m + 1),
                                512 * b : 512 * (b + 1),
                            ],
                            osb[:],
                        )
                    return go

                chunks = []
                for m in range(4):
                    for kp_lo in range(0, 16, 4):
                        chunks.append(mk_mm(m, kp_lo, kp_lo + 4))
                    chunks.append(mk_evac(m))
                return chunks

            def emit_wo(b):
                with (
                    tc.tile_pool(name=f"woo{b}w", bufs=2) as woop,
                    tc.tile_pool(name=f"wops{b}w", bufs=2, space="PSUM") as wops,
                ):
                    for ch in make_wo_chunks(b, woop, wops):
                        ch()

            if adapter_skip:
                emit_qkv(0)
                emit_qkv(1)
                emit_attn2_fast(0, 1, pf_b=2)
                emit_qkv(2)
                emit_qkv(3)
                emit_attn2_fast(2, 3, wo_pf=0)
                emit_wo(0)
                emit_wo(1)
                emit_wo(2)
                emit_wo(3)
            else:
                emit_qkv(0)
                emit_qkv(1)
                emit_attn_generic(0)
                emit_qkv(2)
                emit_attn_generic(1)
                emit_qkv(3)
                emit_attn_generic(2)
                emit_wo(0)
                emit_attn_generic(3)
                emit_wo(1)
                emit_wo(2)
                emit_wo(3)

    nc.compile()
    return nc


def kernel(**inputs) -> np.ndarray:
    in_maps, causal, adapter_skip = _host_prep(inputs)
    key = (causal, adapter_skip)
    if key not in _cache:
        _cache[key] = _build(causal, adapter_skip)
    nc = _cache[key]
    res = run_bass_kernel_spmd(nc, in_maps, core_ids=list(range(NCORES)))
    global last_result
    last_result = res
    out = np.empty((B * S, D), np.float32)
    for r in range(NCORES):
        out[:, 512 * r : 512 * (r + 1)] = res.results[r]["out_r"].T
    return out.reshape(B, S, D)


if __name__ == "__main__":
    rng = np.random.default_rng(0)
    demo = {
        "x": rng.standard_normal((B, S, D), dtype=np.float32),
        "adapter": rng.standard_normal((B, A_LEN, D), dtype=np.float32),
        "mask": np.where(
            np.tril(np.ones((S, S), dtype=bool)), 0.0, -1e9
        ).astype(np.float32)[None, None],
        "freqs_cos": rng.random((S, 64), dtype=np.float32),
        "freqs_sin": rng.random((S, 64), dtype=np.float32),
        "wq": (rng.standard_normal((D, H * HD), dtype=np.float32) * 0.02),
        "wk": (rng.standard_normal((D, HK * HD), dtype=np.float32) * 0.02),
        "wv": (rng.standard_normal((D, HK * HD), dtype=np.float32) * 0.02),
        "wo": (rng.standard_normal((H * HD, D), dtype=np.float32) * 0.02),
        "gate": np.zeros((1, H, 1, 1), np.float32),
    }
    o = kernel(**demo)
    print("kernel ran, out shape", o.shape)
